# revision 1
# baseline (speedup 1.0000x reference)
"""TRN2 Bass/Tile kernel: causal self-attention with RoPE.

Sharding across 8 NeuronCores: batch (2) x head-groups (4 groups of 4 heads,
tensor-parallel). Each core computes, for its batch and its 4 heads:
Q/K/V projections (RoPE folded into doubled Q/K weight matmuls), causal
softmax attention in transposed (scores^T) orientation with the softmax
denominator obtained via an extra ones-column in V, and a partial output
projection. The host sums the 4 partial outputs per batch.

All matmuls run in float32r (TF32-like, full-rate for free dim >= 256,
fp32 PSUM accumulation); measured end-to-end rel error ~3e-4.
"""
import numpy as np
import ml_dtypes
import concourse.bass as bass
from concourse import bacc
import concourse.mybir as mybir
import concourse.tile as tile
from concourse.bass_utils import run_bass_kernel_spmd

B, S, D = 2, 2048, 1024
H, DK = 16, 64
THETA = 10000.0
ST = 512              # q-band / projection s-tile width
NSC = S // 128        # 16 s-chunks of 128
f32 = mybir.dt.float32
f32r = mybir.dt.float32r
bf16 = mybir.dt.bfloat16
AF = mybir.ActivationFunctionType
Alu = mybir.AluOpType

# v_aug layout per s-chunk, per head pair: A head [v(64) | one],
# B head [one | zeros(63) | v(64)] (places attn rows at psum partitions 64:128)
VA = 65
VB = 128
VHP = VA + VB        # 193
VSC = 2 * VHP        # 386

_NC = None
_CONSTS = None


def _build():
    import os
    phases = os.environ.get("K_PHASES", "ABC")
    nc = bacc.Bacc()
    xT = nc.dram_tensor("xT", [D, S], f32r, kind="ExternalInput")
    pw = nc.dram_tensor("pw", [D, 1280], f32r, kind="ExternalInput")
    woT = nc.dram_tensor("woT", [256, D], f32r, kind="ExternalInput")
    cossin = nc.dram_tensor("cossin", [128, 2 * S], f32, kind="ExternalInput")
    masks = nc.dram_tensor("masks", [128, 4096], bf16, kind="ExternalInput")
    vpat = nc.dram_tensor("vpat", [128, NSC * VSC], f32r, kind="ExternalInput")
    onesd = nc.dram_tensor("onesd", [128, 128], f32r, kind="ExternalInput")
    out = nc.dram_tensor("out", [S, D], f32, kind="ExternalOutput")

    with tile.TileContext(nc) as tc:
        with tc.tile_pool(name="persist", bufs=1) as pp:
            qT = [pp.tile([128, S], f32r, tag=f"qT{i}", name=f"qT{i}") for i in range(2)]
            kT = [pp.tile([128, S], f32r, tag=f"kT{i}", name=f"kT{i}") for i in range(2)]
            v_aug = pp.tile([128, NSC * VSC], f32r, tag="vaug")
            concatT = [pp.tile([128, S], f32r, tag=f"cT{i}", name=f"cT{i}") for i in range(2)]
            woT_sb = pp.tile([128, 2, D], f32r, tag="woT")
            ones_sb = pp.tile([128, 128], f32r, tag="ones")
            masks_sb = pp.tile([128, 4096], bf16, tag="masks")

            nc.sync.dma_start(masks_sb[:], masks[:])
            nc.sync.dma_start(ones_sb[:], onesd[:])
            nc.sync.dma_start(v_aug[:], vpat[:])
            nc.sync.dma_start(woT_sb[:],
                              woT[:].rearrange("(k p) m -> p k m", p=128))

            va_sc = v_aug[:].rearrange("p (c h r) -> p c h r", c=NSC, r=VHP)

            # ---- Phase A: projections + RoPE + V ----
            with tc.tile_pool(name="pa", bufs=1) as pa, \
                 tc.tile_pool(name="pax", bufs=2) as pax, \
                 tc.tile_pool(name="prope", bufs=4) as prope, \
                 tc.tile_pool(name="psA", bufs=6, space="PSUM") as psA, \
                 tc.tile_pool(name="psV", bufs=2, space="PSUM") as psV:
                pw_sb = pa.tile([128, 8, 1280], f32r, tag="pw")
                cs_sb = pa.tile([128, 2, S], f32, tag="cs")
                nc.sync.dma_start(pw_sb[:],
                                  pw[:].rearrange("(k p) m -> p k m", p=128))
                nc.sync.dma_start(cs_sb[:],
                                  cossin[:].rearrange("p (c s) -> p c s", c=2))

                for st in range(4):
                    xs = pax.tile([128, 8, ST], f32r, tag="xs")
                    nc.sync.dma_start(
                        xs[:],
                        xT[:, st * ST:(st + 1) * ST]
                        .rearrange("(k p) m -> p k m", p=128))
                    sl = slice(st * ST, (st + 1) * ST)
                    for hp in range(2):
                        for aoff, boff, dst in ((0, 256, qT), (512, 768, kT)):
                            pa_ps = psA.tile([128, ST], f32, tag="proj")
                            pb_ps = psA.tile([128, ST], f32, tag="proj")
                            ao = aoff + 128 * hp
                            bo = boff + 128 * hp
                            for kt in range(8):
                                nc.tensor.matmul(pa_ps[:],
                                                 pw_sb[:, kt, ao:ao + 128],
                                                 xs[:, kt, :],
                                                 start=(kt == 0), stop=(kt == 7))
                            for kt in range(8):
                                nc.tensor.matmul(pb_ps[:],
                                                 pw_sb[:, kt, bo:bo + 128],
                                                 xs[:, kt, :],
                                                 start=(kt == 0), stop=(kt == 7))
                            t1 = prope.tile([128, ST], f32r, tag="ropea")
                            t2 = prope.tile([128, ST], f32r, tag="ropeb")
                            nc.vector.tensor_tensor(t1[:], pa_ps[:],
                                                    cs_sb[:, 0, sl], Alu.mult)
                            nc.vector.tensor_tensor(t2[:], pb_ps[:],
                                                    cs_sb[:, 1, sl], Alu.mult)
                            nc.vector.tensor_tensor(dst[hp][:, sl], t1[:],
                                                    t2[:], Alu.add)
                    for scl in range(4):
                        sc = st * 4 + scl
                        vp = psV.tile([128, 256], f32, tag="vproj")
                        for kt in range(8):
                            nc.tensor.matmul(vp[:],
                                             xs[:, kt, scl * 128:(scl + 1) * 128],
                                             pw_sb[:, kt, 1024:1280],
                                             start=(kt == 0), stop=(kt == 7))
                        vp_r = vp[:].rearrange("p (g t e) -> p g t e", g=2, t=2)
                        nc.vector.tensor_copy(va_sc[:, sc, :, 0:64],
                                              vp_r[:, :, 0, :])
                        nc.vector.tensor_copy(va_sc[:, sc, :, VA + 64:VHP],
                                              vp_r[:, :, 1, :])

            # ---- Phase B: attention (scores^T -> exp -> PV -> normalize) ----
            if "B" not in phases:
                return _finish(nc)
            with tc.tile_pool(name="wtp", bufs=5) as wtp, \
                 tc.tile_pool(name="dnp", bufs=2) as dnp, \
                 tc.tile_pool(name="bcp", bufs=2) as bcp, \
                 tc.tile_pool(name="psS", bufs=2, space="PSUM") as psS, \
                 tc.tile_pool(name="psP", bufs=2, space="PSUM") as psP, \
                 tc.tile_pool(name="psB", bufs=1, space="PSUM") as psB:
                for band in range(4):
                    qsl = slice(band * ST, (band + 1) * ST)
                    nkt = 4 * band + 4
                    # diagonal k-tiles first: their mask multiply runs on
                    # gpsimd while PE/ACT stream the full (unmasked) k-tiles
                    kts = list(range(4 * band, nkt)) + list(range(0, 4 * band))
                    for hp in range(2):
                        pvA = psP.tile([65, ST], f32, tag="pv")
                        pvB = psP.tile([128, ST], f32, tag="pv")
                        for i, kt in enumerate(kts):
                            ksl = slice(kt * 128, (kt + 1) * 128)
                            scp = psS.tile([128, 1024], f32, tag="sc")
                            nc.tensor.matmul(scp[:, 0:512],
                                             kT[hp][0:64, ksl],
                                             qT[hp][0:64, qsl],
                                             start=True, stop=True)
                            nc.tensor.matmul(scp[:, 512:1024],
                                             kT[hp][64:128, ksl],
                                             qT[hp][64:128, qsl],
                                             start=True, stop=True)
                            wt = wtp.tile([128, 1024], f32r, tag="wt")
                            nc.scalar.activation(wt[:], scp[:], AF.Exp,
                                                 scale=0.125)
                            j = kt - 4 * band
                            if j >= 0:
                                eng = nc.vector if band == 0 else nc.gpsimd
                                eng.tensor_tensor(
                                    wt[:], wt[:],
                                    masks_sb[:, j * 1024:(j + 1) * 1024],
                                    Alu.mult)
                            nc.tensor.matmul(pvA[:],
                                             va_sc[:, kt, hp, 0:VA],
                                             wt[:, 0:512],
                                             start=(i == 0),
                                             stop=(i == nkt - 1),
                                             skip_group_check=True)
                            nc.tensor.matmul(pvB[:],
                                             va_sc[:, kt, hp, VA:VHP],
                                             wt[:, 512:1024],
                                             start=(i == 0),
                                             stop=(i == nkt - 1),
                                             skip_group_check=True)
                        # normalize head A (denominator at pvA row 64)
                        dnA = dnp.tile([65, ST], f32r, tag="dna")
                        nc.scalar.copy(dnA[64:65, :], pvA[64:65, :])
                        bcA_ps = psB.tile([64, ST], f32, tag="bca")
                        nc.tensor.matmul(bcA_ps[:], ones_sb[64:65, 0:64],
                                         dnA[64:65, :], start=True, stop=True)
                        bcA = bcp.tile([64, ST], f32, tag="bca")
                        nc.vector.reciprocal_approx_fast(bcA[:], bcA_ps[:])
                        nc.vector.tensor_tensor(concatT[hp][0:64, qsl],
                                                pvA[0:64, :], bcA[:], Alu.mult)
                        # normalize head B (denominator at pvB row 0,
                        # attn rows at 64:128)
                        rB = dnp.tile([65, ST], f32, tag="rb")
                        nc.vector.reciprocal_approx_fast(rB[0:1, :],
                                                         pvB[0:1, :])
                        dnB = dnp.tile([65, ST], f32r, tag="dnb")
                        nc.scalar.copy(dnB[0:1, :], rB[0:1, :])
                        bcB_ps = psB.tile([128, ST], f32, tag="bcb")
                        nc.tensor.matmul(bcB_ps[:], ones_sb[0:1, :],
                                         dnB[0:1, :], start=True, stop=True)
                        bcB = bcp.tile([128, ST], f32, tag="bcb")
                        nc.scalar.copy(bcB[64:128, :], bcB_ps[64:128, :])
                        nc.vector.tensor_tensor(concatT[hp][64:128, qsl],
                                                pvB[64:128, :], bcB[64:128, :],
                                                Alu.mult)

            # ---- Phase C: output projection (partial) ----
            if "C" not in phases:
                return _finish(nc)
            with tc.tile_pool(name="outp", bufs=3) as outp, \
                 tc.tile_pool(name="psO", bufs=2, space="PSUM") as psO:
                for sc in range(NSC):
                    ssl = slice(sc * 128, (sc + 1) * 128)
                    op_ps = psO.tile([128, D], f32, tag="op")
                    for ds in range(2):
                        dsl = slice(ds * 512, (ds + 1) * 512)
                        for ot in range(2):
                            nc.tensor.matmul(op_ps[:, dsl],
                                             concatT[ot][:, ssl],
                                             woT_sb[:, ot, dsl],
                                             start=(ot == 0), stop=(ot == 1))
                    ob = outp.tile([128, D], f32, tag="ob")
                    nc.vector.tensor_copy(ob[:], op_ps[:])
                    nc.sync.dma_start(out[ssl, :], ob[:])
    nc.finalize()
    return nc


def _rope_tables():
    inv_freq = 1.0 / (THETA ** (np.arange(0, DK, 2, dtype=np.float64) / DK))
    t = np.arange(S, dtype=np.float64)
    freqs = np.outer(t, inv_freq)
    emb = np.stack((freqs, freqs), axis=-1).reshape(S, DK)
    return np.cos(emb).astype(np.float32), np.sin(emb).astype(np.float32)


def _sgn_shuf(w):
    ws = np.empty_like(w)
    ws[0::2] = -w[1::2]
    ws[1::2] = w[0::2]
    return ws


def _host_consts():
    f_idx = np.arange(512)
    p_idx = np.arange(128)
    mblocks = []
    for j in range(4):
        mj = (f_idx[None, :] >= p_idx[:, None] + 128 * j).astype(np.float32)
        mblocks.append(np.tile(mj, (1, 2)))
    masks_np = np.concatenate(mblocks, axis=1).astype(ml_dtypes.bfloat16)

    vpat_np = np.zeros((128, NSC * VSC), np.float32)
    for sc in range(NSC):
        for r in range(2):
            base = sc * VSC + r * VHP
            vpat_np[:, base + 64] = 1.0   # A ones column
            vpat_np[:, base + VA] = 1.0   # B ones column

    onesd_np = np.zeros((128, 128), np.float32)
    onesd_np[64, 0:64] = 1.0              # lhsT for head-A broadcast
    onesd_np[0, 64:128] = 1.0             # lhsT for head-B broadcast
    return masks_np, vpat_np, onesd_np


def kernel(x, token_positions, W_q, W_k, W_v, W_o):
    global _NC
    if _NC is None:
        _NC = _build()
    x = np.asarray(x, dtype=np.float32)
    token_positions = np.asarray(token_positions)
    W_q = np.asarray(W_q, dtype=np.float32)
    W_k = np.asarray(W_k, dtype=np.float32)
    W_v = np.asarray(W_v, dtype=np.float32)
    W_o = np.asarray(W_o, dtype=np.float32)

    global _CONSTS
    if _CONSTS is None:
        _CONSTS = (*_rope_tables(), *_host_consts())
    cos_t, sin_t, masks_np, vpat_np, onesd_np = _CONSTS

    in_maps = []
    for c in range(8):
        b, g = divmod(c, 4)
        rows = slice(256 * g, 256 * (g + 1))
        wq, wk, wv = W_q[rows], W_k[rows], W_v[rows]
        pw_np = np.ascontiguousarray(np.concatenate(
            [wq.T, _sgn_shuf(wq).T, wk.T, _sgn_shuf(wk).T, wv.T], axis=1))
        woT_np = np.ascontiguousarray(W_o[:, rows].T)
        pos = np.asarray(token_positions[b], dtype=np.int64)
        cosT = np.tile(cos_t[pos].T, (2, 1))
        sinT = np.tile(sin_t[pos].T, (2, 1))
        cossin_np = np.ascontiguousarray(
            np.concatenate([cosT, sinT], axis=1), dtype=np.float32)
        xT_np = np.ascontiguousarray(x[b].T)
        in_maps.append({
            "xT": xT_np, "pw": pw_np, "woT": woT_np, "cossin": cossin_np,
            "masks": masks_np, "vpat": vpat_np, "onesd": onesd_np,
        })

    res = run_bass_kernel_spmd(_NC, in_maps, core_ids=list(range(8)))
    outs = [res.results[c]["out"] for c in range(8)]
    o0 = outs[0] + outs[1] + outs[2] + outs[3]
    o1 = outs[4] + outs[5] + outs[6] + outs[7]
    return np.stack([o0, o1]).astype(np.float32)



# revision 37
# speedup vs baseline: 1.5735x; 1.5735x over previous
"""TRN2 Bass/Tile kernel: causal self-attention with RoPE.

Sharding across 8 NeuronCores: batch (2) x head-groups (4 groups of 4 heads,
tensor-parallel). Each core computes, for its batch and its 4 heads:
Q/K/V projections in bf16 (RoPE applied via a signed pair-swap permutation
matmul + cos/sin elementwise combine), causal softmax attention in transposed
(scores^T) orientation with the softmax denominator obtained via an extra
ones-column in V, and a partial output projection. The host sums the 4
partial outputs per batch.

All on-chip matmul operands are bf16 with fp32 PSUM accumulation. Phase B is
software-pipelined (scores run 2 k-tiles ahead of the PV accumulation) so the
tensor engine stays busy while the activation engine computes exp().
"""
import numpy as np
import ml_dtypes
import concourse.bass as bass
from concourse import bacc
import concourse.mybir as mybir
import concourse.tile as tile
from concourse.bass_utils import run_bass_kernel_spmd

B, S, D = 2, 2048, 1024
H, DK = 16, 64
THETA = 10000.0
ST = 512              # q-band / projection s-tile width
NSC = S // 128        # 16 s-chunks of 128
f32 = mybir.dt.float32
bf16 = mybir.dt.bfloat16
AF = mybir.ActivationFunctionType
Alu = mybir.AluOpType

# v_aug layout per s-chunk, per head pair: A head [v(64) | one],
# B head [one | zeros(63) | v(64)] (places attn rows at psum partitions 64:128)
VA = 65
VB = 128
VHP = VA + VB        # 193
VSC = 2 * VHP        # 386

_NC = None
_CONSTS = None


def _build():
    import os
    debug = bool(os.environ.get("K_DEBUG"))
    slack = int(os.environ.get("K_SLACK", "2"))
    nofill = bool(os.environ.get("K_NOFILL"))
    nc = bacc.Bacc()
    xT = nc.dram_tensor("xT", [D, S], bf16, kind="ExternalInput")
    pw = nc.dram_tensor("pw", [D, 768], bf16, kind="ExternalInput")
    woT = nc.dram_tensor("woT", [256, D], bf16, kind="ExternalInput")
    cossin = nc.dram_tensor("cossin", [128, 2 * S], bf16, kind="ExternalInput")
    masks = nc.dram_tensor("masks", [128, 4096], bf16, kind="ExternalInput")
    vpat = nc.dram_tensor("vpat", [128, NSC * VSC], bf16, kind="ExternalInput")
    onesd = nc.dram_tensor("onesd", [128, 128], bf16, kind="ExternalInput")
    permd = nc.dram_tensor("permd", [128, 128], bf16, kind="ExternalInput")
    out = nc.dram_tensor("out", [S, D], f32, kind="ExternalOutput")
    if debug:
        dbg = {name: nc.dram_tensor(name, shape, bf16, kind="ExternalOutput")
               for name, shape in [("dq0", [128, S]), ("dk0", [128, S]),
                                   ("dva", [128, NSC * VSC]),
                                   ("dc0", [128, S]), ("dc1", [128, S]),
                                   ("dwt", [128, 1024]), ("dpva", [65, 512]),
                                   ("dpvb", [128, 512]), ("dbca", [64, 512])]}

    with tile.TileContext(nc) as tc:
        with tc.tile_pool(name="persist", bufs=1) as pp:
            qT = [pp.tile([128, S], bf16, tag=f"qT{i}", name=f"qT{i}") for i in range(2)]
            kT = [pp.tile([128, S], bf16, tag=f"kT{i}", name=f"kT{i}") for i in range(2)]
            v_aug = pp.tile([128, NSC * VSC], bf16, tag="vaug")
            concatT = [pp.tile([128, S], bf16, tag=f"cT{i}", name=f"cT{i}") for i in range(2)]
            pw_sb = pp.tile([128, 8, 768], bf16, tag="pw")
            cs_sb = pp.tile([128, 2, S], bf16, tag="cs")
            woT_sb = pp.tile([128, 2, D], bf16, tag="woT")
            ones_sb = pp.tile([128, 128], bf16, tag="ones")
            perm_sb = pp.tile([128, 128], bf16, tag="perm")
            masks_sb = pp.tile([128, 4096], bf16, tag="masks")

            with tc.tile_pool(name="pax", bufs=2) as pax:
                xs = [pax.tile([128, 8, ST], bf16, tag="xs", name=f"xs{i}")
                      for i in range(4)]
                # DMA order = queue order: weights + first x chunk first so the
                # PE can start ~4us in; constants later.
                nc.sync.dma_start(pw_sb[:, 0:4, :],
                                  pw[0:512, :].rearrange("(k p) m -> p k m", p=128))
                nc.sync.dma_start(xs[0][:, 0:4, :],
                                  xT[0:512, 0:ST].rearrange("(k p) m -> p k m", p=128))
                nc.sync.dma_start(pw_sb[:, 4:8, :],
                                  pw[512:1024, :].rearrange("(k p) m -> p k m", p=128))
                nc.sync.dma_start(xs[0][:, 4:8, :],
                                  xT[512:1024, 0:ST].rearrange("(k p) m -> p k m", p=128))
                nc.sync.dma_start(perm_sb[:], permd[:])
                nc.sync.dma_start(xs[1][:],
                                  xT[:, ST:2 * ST].rearrange("(k p) m -> p k m", p=128))
                nc.sync.dma_start(cs_sb[:],
                                  cossin[:].rearrange("p (c s) -> p c s", c=2))
                nc.sync.dma_start(ones_sb[:], onesd[:])
                nc.sync.dma_start(masks_sb[:], masks[:])
                nc.sync.dma_start(woT_sb[:],
                                  woT[:].rearrange("(k p) m -> p k m", p=128))
                nc.sync.dma_start(xs[2][:],
                                  xT[:, 2 * ST:3 * ST].rearrange("(k p) m -> p k m", p=128))
                nc.sync.dma_start(xs[3][:],
                                  xT[:, 3 * ST:4 * ST].rearrange("(k p) m -> p k m", p=128))

                va_sc = v_aug[:].rearrange("p (c h r) -> p c h r", c=NSC, r=VHP)
                # ones columns for the softmax denominator + zero filler
                # around them; the v blocks are overwritten by V copies.
                nc.sync.dma_start(v_aug[:], vpat[:])

                # ---- Phase A: projections + RoPE + V ----
                # Per (st, hp, q/k) tile: 8 accumulation matmuls, ACT copy of
                # the psum to SBUF, pair-swap permutation matmul (emitted one
                # tile later to hide the ACT latency), then q*cos + perm*sin
                # on DVE/Pool. For st==0 the q/k matmuls run kt-major so the
                # PE only needs the first (pw, xs) DMA chunk to start.
                with tc.tile_pool(name="pa", bufs=4, space="PSUM") as pa, \
                     tc.tile_pool(name="prot", bufs=2, space="PSUM") as prot, \
                     tc.tile_pool(name="pvps", bufs=2, space="PSUM") as pvps, \
                     tc.tile_pool(name="pqsb", bufs=3) as pqsb, \
                     tc.tile_pool(name="pt12", bufs=6) as pt12:
                    fillers = []

                    def flush_fillers():
                        while fillers:
                            fillers.pop(0)()

                    def rope_tail(st, t, ps):
                        sl = slice(st * ST, (st + 1) * ST)
                        hp, qk = divmod(t, 2)
                        dst = qT if qk == 0 else kT
                        q_sb = pqsb.tile([128, ST], bf16, tag="qsb")
                        nc.scalar.copy(q_sb[:], ps[:])
                        t1 = pt12.tile([128, ST], bf16, tag="t12")
                        nc.vector.tensor_tensor(t1[:], ps[:],
                                                cs_sb[:, 0, sl], Alu.mult)

                        def mk_perm(hp=hp, dst=dst, sl=sl, q_sb=q_sb, t1=t1):
                            rot = prot.tile([128, ST], f32, tag="rot")
                            nc.tensor.matmul(rot[:], perm_sb[:], q_sb[:],
                                             start=True, stop=True)
                            t2 = pt12.tile([128, ST], bf16, tag="t12")
                            nc.vector.tensor_tensor(t2[:], rot[:],
                                                    cs_sb[:, 1, sl], Alu.mult)
                            nc.gpsimd.tensor_tensor(dst[hp][:, sl], t1[:],
                                                    t2[:], Alu.add)
                        fillers.append(mk_perm)

                    def proj_mm(st, t, ps, kt):
                        hp, qk = divmod(t, 2)
                        off = qk * 256 + 128 * hp
                        nc.tensor.matmul(ps[:],
                                         pw_sb[:, kt, off:off + 128],
                                         xs[st][:, kt, :],
                                         start=(kt == 0), stop=(kt == 7))

                    for st in range(4):
                        if st == 0:
                            pss = [pa.tile([128, ST], f32, tag="proj",
                                           name=f"p0_{t}") for t in range(4)]
                            for kt in range(8):
                                for t in range(4):
                                    proj_mm(0, t, pss[t], kt)
                            for t in range(4):
                                flush_fillers()
                                rope_tail(0, t, pss[t])
                        else:
                            for t in range(4):
                                ps = pa.tile([128, ST], f32, tag="proj")
                                for kt in range(8):
                                    proj_mm(st, t, ps, kt)
                                flush_fillers()
                                rope_tail(st, t, ps)
                        for scl in range(4):
                            sc = st * 4 + scl
                            vp = pvps.tile([128, 256], f32, tag="vproj")
                            for kt in range(8):
                                nc.tensor.matmul(vp[:],
                                                 xs[st][:, kt, scl * 128:(scl + 1) * 128],
                                                 pw_sb[:, kt, 512:768],
                                                 start=(kt == 0), stop=(kt == 7))
                            if scl == 0:
                                flush_fillers()
                            vp_r = vp[:].rearrange("p (g t e) -> p g t e", g=2, t=2)
                            nc.scalar.copy(va_sc[:, sc, :, 0:64], vp_r[:, :, 0, :])
                            nc.scalar.copy(va_sc[:, sc, :, VA + 64:VHP],
                                           vp_r[:, :, 1, :])
                    flush_fillers()

            # ---- Phase B: attention, software-pipelined; Phase C per band ----
            with tc.tile_pool(name="psS", bufs=2, space="PSUM") as psS, \
                 tc.tile_pool(name="psP", bufs=2, space="PSUM") as psP, \
                 tc.tile_pool(name="psO", bufs=2, space="PSUM") as psO, \
                 tc.tile_pool(name="wtp", bufs=3) as wtp, \
                 tc.tile_pool(name="dnp", bufs=4) as dnp, \
                 tc.tile_pool(name="bcp", bufs=4) as bcp, \
                 tc.tile_pool(name="obp", bufs=4) as obp:
                pe_fill = []   # deferred PE work (normalize bcasts, phase C)

                def emit_c_sc(sc, ds):
                    def emit():
                        ssl = slice(sc * 128, (sc + 1) * 128)
                        dsl = slice(ds * 512, (ds + 1) * 512)
                        op = psO.tile([128, 512], f32, tag="oc")
                        nc.tensor.matmul(op[:], concatT[0][:, ssl],
                                         woT_sb[:, 0, dsl],
                                         start=True, stop=False)
                        nc.tensor.matmul(op[:], concatT[1][:, ssl],
                                         woT_sb[:, 1, dsl],
                                         start=False, stop=True)
                        ob = obp.tile([128, 512], f32, tag="ob")
                        if ds == 0:
                            nc.vector.tensor_copy(ob[:], op[:])
                        else:
                            nc.scalar.copy(ob[:], op[:])
                        nc.sync.dma_start(out[ssl, dsl], ob[:])
                    return emit

                for band in range(4):
                    qsl = slice(band * ST, (band + 1) * ST)
                    nkt = 4 * band + 4
                    # diagonal k-tiles first so their mask-multiply overlaps
                    # the unmasked tiles' matmuls
                    kts = list(range(4 * band, nkt)) + list(range(0, 4 * band))
                    for hp in range(2):
                        pvA = psP.tile([65, ST], f32, tag="pv")
                        pvB = psP.tile([128, ST], f32, tag="pv")
                        wts = {}
                        for ii in range(nkt + slack):
                            if ii < nkt:
                                kt = kts[ii]
                                ksl = slice(kt * 128, (kt + 1) * 128)
                                j = kt - 4 * band
                                scp = psS.tile([128, 1024], f32, tag="sc")
                                nc.tensor.matmul(scp[:, 0:512],
                                                 kT[hp][0:64, ksl],
                                                 qT[hp][0:64, qsl],
                                                 start=True, stop=True)
                                nc.tensor.matmul(scp[:, 512:1024],
                                                 kT[hp][64:128, ksl],
                                                 qT[hp][64:128, qsl],
                                                 start=True, stop=True)
                                if pe_fill:
                                    pe_fill.pop(0)()
                                wt = wtp.tile([128, 1024], bf16, tag="wt")
                                nc.scalar.activation(wt[:], scp[:], AF.Exp,
                                                     scale=0.125)
                                if j >= 0:
                                    nc.vector.tensor_tensor(
                                        wt[:], wt[:],
                                        masks_sb[:, j * 1024:(j + 1) * 1024],
                                        Alu.mult)
                                if debug and band == 0 and hp == 0 and ii == 0:
                                    dwt_sb = pp.tile([128, 1024], bf16, tag="dwt")
                                    nc.vector.tensor_copy(dwt_sb[:], wt[:])
                                    nc.sync.dma_start(dbg["dwt"][:], dwt_sb[:])
                                wts[ii] = wt
                            if ii >= slack:
                                i = ii - slack
                                kt = kts[i]
                                wt = wts.pop(i)
                                nc.tensor.matmul(pvA[:],
                                                 va_sc[:, kt, hp, 0:VA],
                                                 wt[:, 0:512],
                                                 start=(i == 0),
                                                 stop=(i == nkt - 1),
                                                 skip_group_check=True)
                                nc.tensor.matmul(pvB[:],
                                                 va_sc[:, kt, hp, VA:VHP],
                                                 wt[:, 512:1024],
                                                 start=(i == 0),
                                                 stop=(i == nkt - 1),
                                                 skip_group_check=True)
                        # normalize: denominators at pvA row 64 (head A) and
                        # pvB row 0 (head B); broadcast bf16 copies via PE,
                        # reciprocal on the broadcast, multiply.
                        # reciprocal_approx_fast only works at partition
                        # offset 0 on hardware, so: head A broadcasts the
                        # denominator to rows 0:64 then recips; head B recips
                        # the [1,512] denominator row then broadcasts.
                        dn = dnp.tile([65, ST], bf16, tag="dn")
                        nc.vector.tensor_copy(dn[64:65, :], pvA[64:65, :])
                        rB = dnp.tile([1, ST], f32, tag="rB")
                        nc.vector.reciprocal_approx_fast(rB[:], pvB[0:1, :])
                        dnB = dnp.tile([1, ST], bf16, tag="dnB")
                        nc.vector.tensor_copy(dnB[:], rB[:])
                        bcA = bcp.tile([64, ST], f32, tag="bcA")
                        bcB = bcp.tile([128, ST], bf16, tag="bcB")

                        def mk_normA(hp=hp, qsl=qsl, pvA=pvA, dn=dn, bcA=bcA):
                            bcA_ps = psO.tile([128, 512], f32, tag="oc")
                            nc.tensor.matmul(bcA_ps[0:64, :], ones_sb[64:65, 0:64],
                                             dn[64:65, :], start=True, stop=True)
                            nc.vector.reciprocal_approx_fast(bcA[:], bcA_ps[0:64, :])
                            nc.vector.tensor_tensor(concatT[hp][0:64, qsl],
                                                    pvA[0:64, :], bcA[:],
                                                    Alu.mult)
                        (mk_normA() if nofill else pe_fill.append(mk_normA))

                        def mk_normB(hp=hp, qsl=qsl, pvB=pvB, dnB=dnB, bcB=bcB):
                            bcB_ps = psO.tile([128, 512], f32, tag="oc")
                            nc.tensor.matmul(bcB_ps[:], ones_sb[0:1, :],
                                             dnB[0:1, :], start=True, stop=True)
                            nc.scalar.copy(bcB[64:128, :], bcB_ps[64:128, :])
                            nc.vector.tensor_tensor(concatT[hp][64:128, qsl],
                                                    pvB[64:128, :], bcB[64:128, :],
                                                    Alu.mult)
                        (mk_normB() if nofill else pe_fill.append(mk_normB))
                        if debug and band == 0 and hp == 0:
                            dpva_sb = pp.tile([65, 512], bf16, tag="dpva")
                            nc.vector.tensor_copy(dpva_sb[:], pvA[:])
                            nc.sync.dma_start(dbg["dpva"][:], dpva_sb[:])
                            dpvb_sb = pp.tile([128, 512], bf16, tag="dpvb")
                            nc.vector.tensor_copy(dpvb_sb[:], pvB[:])
                            nc.sync.dma_start(dbg["dpvb"][:], dpvb_sb[:])

                            def dump_bca(bcA=bcA):
                                dbca_sb = pp.tile([64, 512], bf16, tag="dbca")
                                nc.vector.tensor_copy(dbca_sb[:], bcA[:])
                                nc.sync.dma_start(dbg["dbca"][:], dbca_sb[:])
                            (dump_bca() if nofill else pe_fill.append(dump_bca))
                    for scl in range(4):
                        for ds in range(2):
                            f = emit_c_sc(band * 4 + scl, ds)
                            (f() if nofill else pe_fill.append(f))
                while pe_fill:
                    pe_fill.pop(0)()
            if debug:
                nc.sync.dma_start(dbg["dq0"][:], qT[0][:])
                nc.sync.dma_start(dbg["dk0"][:], kT[0][:])
                nc.sync.dma_start(dbg["dva"][:], v_aug[:])
                nc.sync.dma_start(dbg["dc0"][:], concatT[0][:])
                nc.sync.dma_start(dbg["dc1"][:], concatT[1][:])
    nc.finalize()
    return nc


def _rope_tables():
    inv_freq = 1.0 / (THETA ** (np.arange(0, DK, 2, dtype=np.float64) / DK))
    t = np.arange(S, dtype=np.float64)
    freqs = np.outer(t, inv_freq)
    emb = np.stack((freqs, freqs), axis=-1).reshape(S, DK)
    return np.cos(emb).astype(np.float32), np.sin(emb).astype(np.float32)


def _host_consts():
    f_idx = np.arange(512)
    p_idx = np.arange(128)
    mblocks = []
    for j in range(4):
        mj = (f_idx[None, :] >= p_idx[:, None] + 128 * j).astype(np.float32)
        mblocks.append(np.tile(mj, (1, 2)))
    masks_np = np.concatenate(mblocks, axis=1).astype(ml_dtypes.bfloat16)

    vpat_np = np.zeros((128, NSC * VSC), np.float32)
    for sc in range(NSC):
        for r in range(2):
            base = sc * VSC + r * VHP
            vpat_np[:, base + 64] = 1.0   # A ones column
            vpat_np[:, base + VA] = 1.0   # B ones column
    vpat_np = vpat_np.astype(ml_dtypes.bfloat16)

    onesd_np = np.zeros((128, 128), np.float32)
    onesd_np[64, 0:64] = 1.0              # lhsT for head-A broadcast
    onesd_np[0, 64:128] = 1.0             # lhsT for head-B broadcast

    # signed pair-swap permutation: out[2i] = -q[2i+1], out[2i+1] = q[2i]
    perm_np = np.zeros((128, 128), np.float32)
    ii = np.arange(0, 128, 2)
    perm_np[ii + 1, ii] = -1.0
    perm_np[ii, ii + 1] = 1.0
    return (masks_np, vpat_np, onesd_np.astype(ml_dtypes.bfloat16),
            perm_np.astype(ml_dtypes.bfloat16))


def kernel(x, token_positions, W_q, W_k, W_v, W_o):
    global _NC
    if _NC is None:
        _NC = _build()
    x = np.asarray(x, dtype=np.float32)
    token_positions = np.asarray(token_positions)
    W_q = np.asarray(W_q, dtype=np.float32)
    W_k = np.asarray(W_k, dtype=np.float32)
    W_v = np.asarray(W_v, dtype=np.float32)
    W_o = np.asarray(W_o, dtype=np.float32)

    global _CONSTS
    if _CONSTS is None:
        _CONSTS = (*_rope_tables(), *_host_consts())
    cos_t, sin_t, masks_np, vpat_np, onesd_np, perm_np = _CONSTS

    bf = ml_dtypes.bfloat16
    in_maps = []
    for c in range(8):
        b, g = divmod(c, 4)
        rows = slice(256 * g, 256 * (g + 1))
        pw_np = np.ascontiguousarray(np.concatenate(
            [W_q[rows].T, W_k[rows].T, W_v[rows].T], axis=1)).astype(bf)
        woT_np = np.ascontiguousarray(W_o[:, rows].T).astype(bf)
        pos = np.asarray(token_positions[b], dtype=np.int64)
        cosT = np.tile(cos_t[pos].T, (2, 1))
        sinT = np.tile(sin_t[pos].T, (2, 1))
        cossin_np = np.ascontiguousarray(
            np.concatenate([cosT, sinT], axis=1)).astype(bf)
        xT_np = np.ascontiguousarray(x[b].T).astype(bf)
        in_maps.append({
            "xT": xT_np, "pw": pw_np, "woT": woT_np, "cossin": cossin_np,
            "masks": masks_np, "vpat": vpat_np, "onesd": onesd_np,
            "permd": perm_np,
        })

    res = run_bass_kernel_spmd(_NC, in_maps, core_ids=list(range(8)))
    outs = [res.results[c]["out"] for c in range(8)]
    o0 = outs[0] + outs[1] + outs[2] + outs[3]
    o1 = outs[4] + outs[5] + outs[6] + outs[7]
    return np.stack([o0, o1]).astype(np.float32)


# revision 38
# speedup vs baseline: 1.7174x; 1.0915x over previous
"""TRN2 Bass/Tile kernel: causal self-attention with RoPE.

Sharding across 8 NeuronCores: batch (2) x head-groups (4 groups of 4 heads,
tensor-parallel). Each core computes, for its batch and its 4 heads:
Q/K/V projections in bf16 (RoPE applied via a signed pair-swap permutation
matmul + cos/sin elementwise combine), causal softmax attention in transposed
(scores^T) orientation with the softmax denominator obtained via an extra
ones-column in V, and a partial output projection. The host sums the 4
partial outputs per batch.

All on-chip matmul operands are bf16 with fp32 PSUM accumulation. Phase B is
software-pipelined (scores run 2 k-tiles ahead of the PV accumulation) so the
tensor engine stays busy while the activation engine computes exp().
"""
import numpy as np
import ml_dtypes
import concourse.bass as bass
from concourse import bacc
import concourse.mybir as mybir
import concourse.tile as tile
from concourse.bass_utils import run_bass_kernel_spmd

B, S, D = 2, 2048, 1024
H, DK = 16, 64
THETA = 10000.0
ST = 512              # q-band / projection s-tile width
NSC = S // 128        # 16 s-chunks of 128
f32 = mybir.dt.float32
bf16 = mybir.dt.bfloat16
AF = mybir.ActivationFunctionType
Alu = mybir.AluOpType

# v_aug layout per s-chunk, per head pair: A head [v(64) | one],
# B head [one | zeros(63) | v(64)] (places attn rows at psum partitions 64:128)
VA = 65
VB = 128
VHP = VA + VB        # 193
VSC = 2 * VHP        # 386

_NC = None
_CONSTS = None


def _build():
    import os
    debug = bool(os.environ.get("K_DEBUG"))
    slack = int(os.environ.get("K_SLACK", "2"))
    nofill = bool(os.environ.get("K_NOFILL"))
    nc = bacc.Bacc()
    xT = nc.dram_tensor("xT", [D, S], bf16, kind="ExternalInput")
    pw = nc.dram_tensor("pw", [D, 768], bf16, kind="ExternalInput")
    woT = nc.dram_tensor("woT", [256, D], bf16, kind="ExternalInput")
    cossin = nc.dram_tensor("cossin", [128, 2 * S], bf16, kind="ExternalInput")
    masks = nc.dram_tensor("masks", [128, 256], bf16, kind="ExternalInput")
    vpat = nc.dram_tensor("vpat", [128, NSC * VSC], bf16, kind="ExternalInput")
    onesd = nc.dram_tensor("onesd", [128, 128], bf16, kind="ExternalInput")
    permd = nc.dram_tensor("permd", [128, 128], bf16, kind="ExternalInput")
    out = nc.dram_tensor("out", [S, D], f32, kind="ExternalOutput")
    if debug:
        dbg = {name: nc.dram_tensor(name, shape, bf16, kind="ExternalOutput")
               for name, shape in [("dq0", [128, S]), ("dk0", [128, S]),
                                   ("dva", [128, NSC * VSC]),
                                   ("dc0", [128, S]), ("dc1", [128, S]),
                                   ("dwt", [128, 1024]), ("dpva", [65, 512]),
                                   ("dpvb", [128, 512]), ("dbca", [64, 512])]}

    with tile.TileContext(nc) as tc:
        with tc.tile_pool(name="persist", bufs=1) as pp:
            qT = [pp.tile([128, S], bf16, tag=f"qT{i}", name=f"qT{i}") for i in range(2)]
            kT = [pp.tile([128, S], bf16, tag=f"kT{i}", name=f"kT{i}") for i in range(2)]
            v_aug = pp.tile([128, NSC * VSC], bf16, tag="vaug")
            concatT = [pp.tile([128, S], bf16, tag=f"cT{i}", name=f"cT{i}") for i in range(2)]
            pw_sb = pp.tile([128, 8, 768], bf16, tag="pw")
            cs_sb = pp.tile([128, 2, S], bf16, tag="cs")
            woT_sb = pp.tile([128, 2, D], bf16, tag="woT")
            ones_sb = pp.tile([128, 128], bf16, tag="ones")
            perm_sb = pp.tile([128, 128], bf16, tag="perm")
            masks_sb = pp.tile([128, 256], bf16, tag="masks")

            with tc.tile_pool(name="pax", bufs=2) as pax:
                xs = [pax.tile([128, 8, ST], bf16, tag="xs", name=f"xs{i}")
                      for i in range(4)]
                # DMA order = queue order: weights + first x chunk first so the
                # PE can start ~4us in; constants later.
                nc.sync.dma_start(pw_sb[:, 0:4, :],
                                  pw[0:512, :].rearrange("(k p) m -> p k m", p=128))
                nc.sync.dma_start(xs[0][:, 0:4, :],
                                  xT[0:512, 0:ST].rearrange("(k p) m -> p k m", p=128))
                nc.sync.dma_start(pw_sb[:, 4:8, :],
                                  pw[512:1024, :].rearrange("(k p) m -> p k m", p=128))
                nc.sync.dma_start(xs[0][:, 4:8, :],
                                  xT[512:1024, 0:ST].rearrange("(k p) m -> p k m", p=128))
                nc.sync.dma_start(perm_sb[:], permd[:])
                nc.sync.dma_start(xs[1][:],
                                  xT[:, ST:2 * ST].rearrange("(k p) m -> p k m", p=128))
                nc.sync.dma_start(cs_sb[:],
                                  cossin[:].rearrange("p (c s) -> p c s", c=2))
                nc.sync.dma_start(ones_sb[:], onesd[:])
                nc.sync.dma_start(masks_sb[:], masks[:])
                nc.sync.dma_start(woT_sb[:],
                                  woT[:].rearrange("(k p) m -> p k m", p=128))
                nc.sync.dma_start(xs[2][:],
                                  xT[:, 2 * ST:3 * ST].rearrange("(k p) m -> p k m", p=128))
                nc.sync.dma_start(xs[3][:],
                                  xT[:, 3 * ST:4 * ST].rearrange("(k p) m -> p k m", p=128))

                va_sc = v_aug[:].rearrange("p (c h r) -> p c h r", c=NSC, r=VHP)
                # ones columns for the softmax denominator + zero filler
                # around them; the v blocks are overwritten by V copies.
                nc.sync.dma_start(v_aug[:], vpat[:])

                # ---- Phase A: projections + RoPE + V ----
                # Per (st, hp, q/k) tile: 8 accumulation matmuls, ACT copy of
                # the psum to SBUF, pair-swap permutation matmul (emitted one
                # tile later to hide the ACT latency), then q*cos + perm*sin
                # on DVE/Pool. For st==0 the q/k matmuls run kt-major so the
                # PE only needs the first (pw, xs) DMA chunk to start.
                with tc.tile_pool(name="pa", bufs=4, space="PSUM") as pa, \
                     tc.tile_pool(name="prot", bufs=2, space="PSUM") as prot, \
                     tc.tile_pool(name="pvps", bufs=2, space="PSUM") as pvps, \
                     tc.tile_pool(name="pqsb", bufs=3) as pqsb, \
                     tc.tile_pool(name="pt12", bufs=6) as pt12:
                    fillers = []

                    def flush_fillers():
                        while fillers:
                            fillers.pop(0)()

                    def rope_tail(st, t, ps):
                        sl = slice(st * ST, (st + 1) * ST)
                        hp, qk = divmod(t, 2)
                        dst = qT if qk == 0 else kT
                        q_sb = pqsb.tile([128, ST], bf16, tag="qsb")
                        nc.scalar.copy(q_sb[:], ps[:])
                        t1 = pt12.tile([128, ST], bf16, tag="t12")
                        nc.vector.tensor_tensor(t1[:], ps[:],
                                                cs_sb[:, 0, sl], Alu.mult)

                        def mk_perm(hp=hp, dst=dst, sl=sl, q_sb=q_sb, t1=t1):
                            rot = prot.tile([128, ST], f32, tag="rot")
                            nc.tensor.matmul(rot[:], perm_sb[:], q_sb[:],
                                             start=True, stop=True)
                            t2 = pt12.tile([128, ST], bf16, tag="t12")
                            nc.vector.tensor_tensor(t2[:], rot[:],
                                                    cs_sb[:, 1, sl], Alu.mult)
                            nc.gpsimd.tensor_tensor(dst[hp][:, sl], t1[:],
                                                    t2[:], Alu.add)
                        fillers.append(mk_perm)

                    def proj_mm(st, t, ps, kt):
                        hp, qk = divmod(t, 2)
                        off = qk * 256 + 128 * hp
                        nc.tensor.matmul(ps[:],
                                         pw_sb[:, kt, off:off + 128],
                                         xs[st][:, kt, :],
                                         start=(kt == 0), stop=(kt == 7))

                    for st in range(4):
                        if st == 0:
                            pss = [pa.tile([128, ST], f32, tag="proj",
                                           name=f"p0_{t}") for t in range(4)]
                            for kt in range(8):
                                for t in range(4):
                                    proj_mm(0, t, pss[t], kt)
                            for t in range(4):
                                flush_fillers()
                                rope_tail(0, t, pss[t])
                        else:
                            for t in range(4):
                                ps = pa.tile([128, ST], f32, tag="proj")
                                for kt in range(8):
                                    proj_mm(st, t, ps, kt)
                                flush_fillers()
                                rope_tail(st, t, ps)
                        for scl in range(4):
                            sc = st * 4 + scl
                            vp = pvps.tile([128, 256], f32, tag="vproj")
                            for kt in range(8):
                                nc.tensor.matmul(vp[:],
                                                 xs[st][:, kt, scl * 128:(scl + 1) * 128],
                                                 pw_sb[:, kt, 512:768],
                                                 start=(kt == 0), stop=(kt == 7))
                            if scl == 0:
                                flush_fillers()
                            vp_r = vp[:].rearrange("p (g t e) -> p g t e", g=2, t=2)
                            nc.scalar.copy(va_sc[:, sc, :, 0:64], vp_r[:, :, 0, :])
                            nc.scalar.copy(va_sc[:, sc, :, VA + 64:VHP],
                                           vp_r[:, :, 1, :])
                    flush_fillers()

            # ---- Phase B: attention, software-pipelined; Phase C per band ----
            with tc.tile_pool(name="psS", bufs=2, space="PSUM") as psS, \
                 tc.tile_pool(name="psP", bufs=2, space="PSUM") as psP, \
                 tc.tile_pool(name="psO", bufs=2, space="PSUM") as psO, \
                 tc.tile_pool(name="wtp", bufs=3) as wtp, \
                 tc.tile_pool(name="dnp", bufs=4) as dnp, \
                 tc.tile_pool(name="bcp", bufs=4) as bcp, \
                 tc.tile_pool(name="obp", bufs=4) as obp:
                pe_fill = []   # deferred PE work (normalize bcasts, phase C)
                # dedicated weight tiles for diagonal k-tiles j=1..3: zeroed
                # once; exp only ever rewrites the causally-needed columns,
                # so the masked-out ranges stay zero.
                wt_d = {j: pp.tile([128, 1024], bf16, tag=f"wtd{j}",
                                   name=f"wtd{j}") for j in (1, 2, 3)}
                for j in (1, 2, 3):
                    nc.gpsimd.memset(wt_d[j][:], 0.0)

                def emit_c_sc(sc, ds):
                    def emit():
                        ssl = slice(sc * 128, (sc + 1) * 128)
                        dsl = slice(ds * 512, (ds + 1) * 512)
                        op = psO.tile([128, 512], f32, tag="oc")
                        nc.tensor.matmul(op[:], concatT[0][:, ssl],
                                         woT_sb[:, 0, dsl],
                                         start=True, stop=False)
                        nc.tensor.matmul(op[:], concatT[1][:, ssl],
                                         woT_sb[:, 1, dsl],
                                         start=False, stop=True)
                        ob = obp.tile([128, 512], f32, tag="ob")
                        if ds == 0:
                            nc.vector.tensor_copy(ob[:], op[:])
                        else:
                            nc.scalar.copy(ob[:], op[:])
                        nc.sync.dma_start(out[ssl, dsl], ob[:])
                    return emit

                for band in range(4):
                    qsl = slice(band * ST, (band + 1) * ST)
                    nkt = 4 * band + 4
                    # diagonal k-tiles first so their mask-multiply overlaps
                    # the unmasked tiles' matmuls
                    kts = list(range(4 * band, nkt)) + list(range(0, 4 * band))
                    for hp in range(2):
                        pvA = psP.tile([65, ST], f32, tag="pv")
                        pvB = psP.tile([128, ST], f32, tag="pv")
                        wts = {}
                        for ii in range(nkt + slack):
                            if ii < nkt:
                                kt = kts[ii]
                                ksl = slice(kt * 128, (kt + 1) * 128)
                                j = kt - 4 * band
                                jc = max(j, 0) * 128   # cropped column offset
                                qcs = slice(qsl.start + jc, qsl.stop)
                                scp = psS.tile([128, 1024], f32, tag="sc")
                                nc.tensor.matmul(scp[:, jc:512],
                                                 kT[hp][0:64, ksl],
                                                 qT[hp][0:64, qcs],
                                                 start=True, stop=True)
                                nc.tensor.matmul(scp[:, 512 + jc:1024],
                                                 kT[hp][64:128, ksl],
                                                 qT[hp][64:128, qcs],
                                                 start=True, stop=True)
                                if pe_fill:
                                    pe_fill.pop(0)()
                                if j >= 1:
                                    wt = wt_d[j]
                                    nc.scalar.activation(wt[:, jc:512],
                                                         scp[:, jc:512],
                                                         AF.Exp, scale=0.125)
                                    nc.scalar.activation(wt[:, 512 + jc:1024],
                                                         scp[:, 512 + jc:1024],
                                                         AF.Exp, scale=0.125)
                                else:
                                    wt = wtp.tile([128, 1024], bf16, tag="wt")
                                    nc.scalar.activation(wt[:], scp[:], AF.Exp,
                                                         scale=0.125)
                                if j >= 0:
                                    nc.vector.tensor_tensor(
                                        wt[:, jc:jc + 128],
                                        wt[:, jc:jc + 128],
                                        masks_sb[:, 0:128], Alu.mult)
                                    nc.vector.tensor_tensor(
                                        wt[:, 512 + jc:512 + jc + 128],
                                        wt[:, 512 + jc:512 + jc + 128],
                                        masks_sb[:, 128:256], Alu.mult)
                                if debug and band == 0 and hp == 0 and ii == 0:
                                    dwt_sb = pp.tile([128, 1024], bf16, tag="dwt")
                                    nc.vector.tensor_copy(dwt_sb[:], wt[:])
                                    nc.sync.dma_start(dbg["dwt"][:], dwt_sb[:])
                                wts[ii] = (wt, jc)
                            if ii >= slack:
                                i = ii - slack
                                kt = kts[i]
                                wt, pjc = wts.pop(i)
                                nc.tensor.matmul(pvA[:, pjc:512],
                                                 va_sc[:, kt, hp, 0:VA],
                                                 wt[:, pjc:512],
                                                 start=(i == 0),
                                                 stop=(i == nkt - 1),
                                                 skip_group_check=True)
                                nc.tensor.matmul(pvB[:, pjc:512],
                                                 va_sc[:, kt, hp, VA:VHP],
                                                 wt[:, 512 + pjc:1024],
                                                 start=(i == 0),
                                                 stop=(i == nkt - 1),
                                                 skip_group_check=True)
                        # normalize: denominators at pvA row 64 (head A) and
                        # pvB row 0 (head B); broadcast bf16 copies via PE,
                        # reciprocal on the broadcast, multiply.
                        # reciprocal_approx_fast only works at partition
                        # offset 0 on hardware, so: head A broadcasts the
                        # denominator to rows 0:64 then recips; head B recips
                        # the [1,512] denominator row then broadcasts.
                        dn = dnp.tile([65, ST], bf16, tag="dn")
                        nc.vector.tensor_copy(dn[64:65, :], pvA[64:65, :])
                        rB = dnp.tile([1, ST], f32, tag="rB")
                        nc.vector.reciprocal_approx_fast(rB[:], pvB[0:1, :])
                        dnB = dnp.tile([1, ST], bf16, tag="dnB")
                        nc.vector.tensor_copy(dnB[:], rB[:])
                        bcA = bcp.tile([64, ST], f32, tag="bcA")
                        bcB = bcp.tile([128, ST], bf16, tag="bcB")

                        def mk_normA(hp=hp, qsl=qsl, pvA=pvA, dn=dn, bcA=bcA):
                            bcA_ps = psO.tile([128, 512], f32, tag="oc")
                            nc.tensor.matmul(bcA_ps[0:64, :], ones_sb[64:65, 0:64],
                                             dn[64:65, :], start=True, stop=True)
                            nc.vector.reciprocal_approx_fast(bcA[:], bcA_ps[0:64, :])
                            nc.vector.tensor_tensor(concatT[hp][0:64, qsl],
                                                    pvA[0:64, :], bcA[:],
                                                    Alu.mult)
                        (mk_normA() if nofill else pe_fill.append(mk_normA))

                        def mk_normB(hp=hp, qsl=qsl, pvB=pvB, dnB=dnB, bcB=bcB):
                            bcB_ps = psO.tile([128, 512], f32, tag="oc")
                            nc.tensor.matmul(bcB_ps[:], ones_sb[0:1, :],
                                             dnB[0:1, :], start=True, stop=True)
                            nc.scalar.copy(bcB[64:128, :], bcB_ps[64:128, :])
                            nc.vector.tensor_tensor(concatT[hp][64:128, qsl],
                                                    pvB[64:128, :], bcB[64:128, :],
                                                    Alu.mult)
                        (mk_normB() if nofill else pe_fill.append(mk_normB))
                        if debug and band == 0 and hp == 0:
                            dpva_sb = pp.tile([65, 512], bf16, tag="dpva")
                            nc.vector.tensor_copy(dpva_sb[:], pvA[:])
                            nc.sync.dma_start(dbg["dpva"][:], dpva_sb[:])
                            dpvb_sb = pp.tile([128, 512], bf16, tag="dpvb")
                            nc.vector.tensor_copy(dpvb_sb[:], pvB[:])
                            nc.sync.dma_start(dbg["dpvb"][:], dpvb_sb[:])

                            def dump_bca(bcA=bcA):
                                dbca_sb = pp.tile([64, 512], bf16, tag="dbca")
                                nc.vector.tensor_copy(dbca_sb[:], bcA[:])
                                nc.sync.dma_start(dbg["dbca"][:], dbca_sb[:])
                            (dump_bca() if nofill else pe_fill.append(dump_bca))
                    for scl in range(4):
                        for ds in range(2):
                            f = emit_c_sc(band * 4 + scl, ds)
                            (f() if nofill else pe_fill.append(f))
                while pe_fill:
                    pe_fill.pop(0)()
            if debug:
                nc.sync.dma_start(dbg["dq0"][:], qT[0][:])
                nc.sync.dma_start(dbg["dk0"][:], kT[0][:])
                nc.sync.dma_start(dbg["dva"][:], v_aug[:])
                nc.sync.dma_start(dbg["dc0"][:], concatT[0][:])
                nc.sync.dma_start(dbg["dc1"][:], concatT[1][:])
    nc.finalize()
    return nc


def _rope_tables():
    inv_freq = 1.0 / (THETA ** (np.arange(0, DK, 2, dtype=np.float64) / DK))
    t = np.arange(S, dtype=np.float64)
    freqs = np.outer(t, inv_freq)
    emb = np.stack((freqs, freqs), axis=-1).reshape(S, DK)
    return np.cos(emb).astype(np.float32), np.sin(emb).astype(np.float32)


def _host_consts():
    # triangle keep-mask for the diagonal 128x128 subtile: keep k_l <= q_s
    f_idx = np.arange(128)
    p_idx = np.arange(128)
    mj = (f_idx[None, :] >= p_idx[:, None]).astype(np.float32)
    masks_np = np.tile(mj, (1, 2)).astype(ml_dtypes.bfloat16)

    vpat_np = np.zeros((128, NSC * VSC), np.float32)
    for sc in range(NSC):
        for r in range(2):
            base = sc * VSC + r * VHP
            vpat_np[:, base + 64] = 1.0   # A ones column
            vpat_np[:, base + VA] = 1.0   # B ones column
    vpat_np = vpat_np.astype(ml_dtypes.bfloat16)

    onesd_np = np.zeros((128, 128), np.float32)
    onesd_np[64, 0:64] = 1.0              # lhsT for head-A broadcast
    onesd_np[0, 64:128] = 1.0             # lhsT for head-B broadcast

    # signed pair-swap permutation: out[2i] = -q[2i+1], out[2i+1] = q[2i]
    perm_np = np.zeros((128, 128), np.float32)
    ii = np.arange(0, 128, 2)
    perm_np[ii + 1, ii] = -1.0
    perm_np[ii, ii + 1] = 1.0
    return (masks_np, vpat_np, onesd_np.astype(ml_dtypes.bfloat16),
            perm_np.astype(ml_dtypes.bfloat16))


def kernel(x, token_positions, W_q, W_k, W_v, W_o):
    global _NC
    if _NC is None:
        _NC = _build()
    x = np.asarray(x, dtype=np.float32)
    token_positions = np.asarray(token_positions)
    W_q = np.asarray(W_q, dtype=np.float32)
    W_k = np.asarray(W_k, dtype=np.float32)
    W_v = np.asarray(W_v, dtype=np.float32)
    W_o = np.asarray(W_o, dtype=np.float32)

    global _CONSTS
    if _CONSTS is None:
        _CONSTS = (*_rope_tables(), *_host_consts())
    cos_t, sin_t, masks_np, vpat_np, onesd_np, perm_np = _CONSTS

    bf = ml_dtypes.bfloat16
    in_maps = []
    for c in range(8):
        b, g = divmod(c, 4)
        rows = slice(256 * g, 256 * (g + 1))
        pw_np = np.ascontiguousarray(np.concatenate(
            [W_q[rows].T, W_k[rows].T, W_v[rows].T], axis=1)).astype(bf)
        woT_np = np.ascontiguousarray(W_o[:, rows].T).astype(bf)
        pos = np.asarray(token_positions[b], dtype=np.int64)
        cosT = np.tile(cos_t[pos].T, (2, 1))
        sinT = np.tile(sin_t[pos].T, (2, 1))
        cossin_np = np.ascontiguousarray(
            np.concatenate([cosT, sinT], axis=1)).astype(bf)
        xT_np = np.ascontiguousarray(x[b].T).astype(bf)
        in_maps.append({
            "xT": xT_np, "pw": pw_np, "woT": woT_np, "cossin": cossin_np,
            "masks": masks_np, "vpat": vpat_np, "onesd": onesd_np,
            "permd": perm_np,
        })

    res = run_bass_kernel_spmd(_NC, in_maps, core_ids=list(range(8)))
    outs = [res.results[c]["out"] for c in range(8)]
    o0 = outs[0] + outs[1] + outs[2] + outs[3]
    o1 = outs[4] + outs[5] + outs[6] + outs[7]
    return np.stack([o0, o1]).astype(np.float32)


# revision 49
# speedup vs baseline: 1.7773x; 1.0349x over previous
"""TRN2 Bass/Tile kernel: causal self-attention with RoPE.

Sharding across 8 NeuronCores: batch (2) x head-groups (4 groups of 4 heads,
tensor-parallel). Each core computes, for its batch and its 4 heads:
Q/K/V projections in bf16 (RoPE applied via a signed pair-swap permutation
matmul + cos/sin elementwise combine), causal softmax attention in transposed
(scores^T) orientation with the softmax denominator obtained via an extra
ones-column in V, and a partial output projection. The host sums the 4
partial outputs per batch.

All on-chip matmul operands are bf16 with fp32 PSUM accumulation. Phase B is
software-pipelined (scores run 2 k-tiles ahead of the PV accumulation) so the
tensor engine stays busy while the activation engine computes exp().
"""
import numpy as np
import ml_dtypes
import concourse.bass as bass
from concourse import bacc
import concourse.mybir as mybir
import concourse.tile as tile
from concourse.bass_utils import run_bass_kernel_spmd

B, S, D = 2, 2048, 1024
H, DK = 16, 64
THETA = 10000.0
ST = 512              # q-band / projection s-tile width
NSC = S // 128        # 16 s-chunks of 128
f32 = mybir.dt.float32
bf16 = mybir.dt.bfloat16
AF = mybir.ActivationFunctionType
Alu = mybir.AluOpType

# v_aug layout per s-chunk, per head pair: A head [v(64) | one],
# B head [one | zeros(63) | v(64)] (places attn rows at psum partitions 64:128)
VA = 65
VB = 128
VHP = VA + VB        # 193
VSC = 2 * VHP        # 386

_NC = None
_CONSTS = None


def _build():
    import os
    debug = bool(os.environ.get("K_DEBUG"))
    slack = int(os.environ.get("K_SLACK", "2"))
    nofill = bool(os.environ.get("K_NOFILL"))
    nc = bacc.Bacc()
    xT = nc.dram_tensor("xT", [D, S], bf16, kind="ExternalInput")
    pw = nc.dram_tensor("pw", [D, 768], bf16, kind="ExternalInput")
    woT = nc.dram_tensor("woT", [256, D], bf16, kind="ExternalInput")
    cossin = nc.dram_tensor("cossin", [128, 2 * S], bf16, kind="ExternalInput")
    masks = nc.dram_tensor("masks", [128, 256], bf16, kind="ExternalInput")
    vpat = nc.dram_tensor("vpat", [128, NSC * VSC], bf16, kind="ExternalInput")
    onesd = nc.dram_tensor("onesd", [128, 128], bf16, kind="ExternalInput")
    permd = nc.dram_tensor("permd", [128, 128], bf16, kind="ExternalInput")
    out = nc.dram_tensor("out", [S, D], bf16, kind="ExternalOutput")
    if debug:
        dbg = {name: nc.dram_tensor(name, shape, bf16, kind="ExternalOutput")
               for name, shape in [("dq0", [128, S]), ("dk0", [128, S]),
                                   ("dva", [128, NSC * VSC]),
                                   ("dc0", [128, S]), ("dc1", [128, S]),
                                   ("dwt", [128, 1024]), ("dpva", [65, 512]),
                                   ("dpvb", [128, 512]), ("dbca", [64, 512])]}

    with tile.TileContext(nc) as tc:
        with tc.tile_pool(name="persist", bufs=1) as pp:
            qT = [pp.tile([128, S], bf16, tag=f"qT{i}", name=f"qT{i}") for i in range(2)]
            kT = [pp.tile([128, S], bf16, tag=f"kT{i}", name=f"kT{i}") for i in range(2)]
            v_aug = pp.tile([128, NSC * VSC], bf16, tag="vaug")
            concatT = [pp.tile([128, S], bf16, tag=f"cT{i}", name=f"cT{i}") for i in range(2)]
            pw_sb = pp.tile([128, 8, 768], bf16, tag="pw")
            cs_sb = pp.tile([128, 2, S], bf16, tag="cs")
            woT_sb = pp.tile([128, 2, D], bf16, tag="woT")
            ones_sb = pp.tile([128, 128], bf16, tag="ones")
            perm_sb = pp.tile([128, 128], bf16, tag="perm")
            masks_sb = pp.tile([128, 256], bf16, tag="masks")
            ones_f32 = pp.tile([1, 128], f32, tag="ones_f32")

            with tc.tile_pool(name="pax", bufs=2) as pax:
                xs = [pax.tile([128, 8, ST], bf16, tag="xs", name=f"xs{i}")
                      for i in range(4)]
                # DMA order = queue order: weights + first x chunk first so the
                # PE can start ~4us in; constants later.
                nc.sync.dma_start(pw_sb[:, 0:2, :],
                                  pw[0:256, :].rearrange("(k p) m -> p k m", p=128))
                nc.sync.dma_start(xs[0][:, 0:2, :],
                                  xT[0:256, 0:ST].rearrange("(k p) m -> p k m", p=128))
                nc.sync.dma_start(pw_sb[:, 2:4, :],
                                  pw[256:512, :].rearrange("(k p) m -> p k m", p=128))
                nc.sync.dma_start(xs[0][:, 2:4, :],
                                  xT[256:512, 0:ST].rearrange("(k p) m -> p k m", p=128))
                nc.sync.dma_start(pw_sb[:, 4:8, :],
                                  pw[512:1024, :].rearrange("(k p) m -> p k m", p=128))
                nc.sync.dma_start(xs[0][:, 4:8, :],
                                  xT[512:1024, 0:ST].rearrange("(k p) m -> p k m", p=128))
                nc.sync.dma_start(perm_sb[:], permd[:])
                nc.sync.dma_start(xs[1][:],
                                  xT[:, ST:2 * ST].rearrange("(k p) m -> p k m", p=128))
                nc.sync.dma_start(cs_sb[:],
                                  cossin[:].rearrange("p (c s) -> p c s", c=2))
                nc.sync.dma_start(ones_sb[:], onesd[:])
                nc.sync.dma_start(masks_sb[:], masks[:])
                nc.sync.dma_start(woT_sb[:],
                                  woT[:].rearrange("(k p) m -> p k m", p=128))
                nc.sync.dma_start(xs[2][:],
                                  xT[:, 2 * ST:3 * ST].rearrange("(k p) m -> p k m", p=128))
                nc.sync.dma_start(xs[3][:],
                                  xT[:, 3 * ST:4 * ST].rearrange("(k p) m -> p k m", p=128))

                nc.gpsimd.memset(ones_f32[:], 1.0)
                va_sc = v_aug[:].rearrange("p (c h r) -> p c h r", c=NSC, r=VHP)
                # ones columns for the softmax denominator + zero filler
                # around them; the v blocks are overwritten by V copies.
                nc.sync.dma_start(v_aug[:], vpat[:])

                # ---- Phase A: projections + RoPE + V ----
                # Per (st, hp, q/k) tile: 8 accumulation matmuls, ACT copy of
                # the psum to SBUF, pair-swap permutation matmul (emitted one
                # tile later to hide the ACT latency), then q*cos + perm*sin
                # on DVE/Pool. For st==0 the q/k matmuls run kt-major so the
                # PE only needs the first (pw, xs) DMA chunk to start.
                with tc.tile_pool(name="pa", bufs=4, space="PSUM") as pa, \
                     tc.tile_pool(name="prot", bufs=2, space="PSUM") as prot, \
                     tc.tile_pool(name="pvps", bufs=2, space="PSUM") as pvps, \
                     tc.tile_pool(name="pqsb", bufs=3) as pqsb, \
                     tc.tile_pool(name="pt12", bufs=6) as pt12:
                    fillers = []

                    def flush_fillers():
                        while fillers:
                            fillers.pop(0)()

                    def rope_tail(st, t, ps):
                        sl = slice(st * ST, (st + 1) * ST)
                        hp, qk = divmod(t, 2)
                        dst = qT if qk == 0 else kT
                        q_sb = pqsb.tile([128, ST], bf16, tag="qsb")
                        nc.scalar.copy(q_sb[:], ps[:])
                        t1 = pt12.tile([128, ST], bf16, tag="t12")
                        nc.vector.tensor_tensor(t1[:], ps[:],
                                                cs_sb[:, 0, sl], Alu.mult)

                        def mk_perm(hp=hp, dst=dst, sl=sl, q_sb=q_sb, t1=t1):
                            rot = prot.tile([128, ST], f32, tag="rot")
                            nc.tensor.matmul(rot[:], perm_sb[:], q_sb[:],
                                             start=True, stop=True)
                            t2 = pt12.tile([128, ST], bf16, tag="t12")
                            nc.vector.tensor_tensor(t2[:], rot[:],
                                                    cs_sb[:, 1, sl], Alu.mult)
                            nc.gpsimd.tensor_tensor(dst[hp][:, sl], t1[:],
                                                    t2[:], Alu.add)
                        fillers.append(mk_perm)

                    def proj_mm(st, t, ps, kt):
                        hp, qk = divmod(t, 2)
                        off = qk * 256 + 128 * hp
                        nc.tensor.matmul(ps[:],
                                         pw_sb[:, kt, off:off + 128],
                                         xs[st][:, kt, :],
                                         start=(kt == 0), stop=(kt == 7))

                    for st in range(4):
                        if st == 0:
                            pss = [pa.tile([128, ST], f32, tag="proj",
                                           name=f"p0_{t}") for t in range(4)]
                            for kt in range(8):
                                for t in range(4):
                                    proj_mm(0, t, pss[t], kt)
                            for t in range(4):
                                flush_fillers()
                                rope_tail(0, t, pss[t])
                        else:
                            for t in range(4):
                                ps = pa.tile([128, ST], f32, tag="proj")
                                for kt in range(8):
                                    proj_mm(st, t, ps, kt)
                                flush_fillers()
                                rope_tail(st, t, ps)
                        for scl in range(4):
                            sc = st * 4 + scl
                            vp = pvps.tile([128, 256], f32, tag="vproj")
                            for kt in range(8):
                                nc.tensor.matmul(vp[:],
                                                 xs[st][:, kt, scl * 128:(scl + 1) * 128],
                                                 pw_sb[:, kt, 512:768],
                                                 start=(kt == 0), stop=(kt == 7))
                            if scl == 0:
                                flush_fillers()
                            vp_r = vp[:].rearrange("p (g t e) -> p g t e", g=2, t=2)
                            nc.scalar.copy(va_sc[:, sc, :, 0:64], vp_r[:, :, 0, :])
                            nc.scalar.copy(va_sc[:, sc, :, VA + 64:VHP],
                                           vp_r[:, :, 1, :])
                    flush_fillers()

            # ---- Phase B: attention, software-pipelined; Phase C per band ----
            with tc.tile_pool(name="psS", bufs=2, space="PSUM") as psS, \
                 tc.tile_pool(name="psP", bufs=2, space="PSUM") as psP, \
                 tc.tile_pool(name="psO", bufs=2, space="PSUM") as psO, \
                 tc.tile_pool(name="wtp", bufs=3) as wtp, \
                 tc.tile_pool(name="dnp", bufs=4) as dnp, \
                 tc.tile_pool(name="bcp", bufs=4) as bcp, \
                 tc.tile_pool(name="obp", bufs=4) as obp:
                pe_fill = []   # deferred PE work (normalize bcasts, phase C)
                # dedicated weight tiles for diagonal k-tiles j=1..3: zeroed
                # once; exp only ever rewrites the causally-needed columns,
                # so the masked-out ranges stay zero.
                wt_d = {j: pp.tile([128, 1024], bf16, tag=f"wtd{j}",
                                   name=f"wtd{j}") for j in (1, 2, 3)}
                for j in (1, 2, 3):
                    nc.gpsimd.memset(wt_d[j][:], 0.0)

                def emit_c_sc(sc, ds):
                    def emit():
                        ssl = slice(sc * 128, (sc + 1) * 128)
                        dsl = slice(ds * 512, (ds + 1) * 512)
                        op = psO.tile([128, 512], f32, tag="oc")
                        nc.tensor.matmul(op[:], concatT[0][:, ssl],
                                         woT_sb[:, 0, dsl],
                                         start=True, stop=False)
                        nc.tensor.matmul(op[:], concatT[1][:, ssl],
                                         woT_sb[:, 1, dsl],
                                         start=False, stop=True)
                        ob = obp.tile([128, 512], bf16, tag="ob")
                        nc.vector.tensor_copy(ob[:], op[:])
                        nc.sync.dma_start(out[ssl, dsl], ob[:])
                    return emit

                for band in range(4):
                    qsl = slice(band * ST, (band + 1) * ST)
                    nkt = 4 * band + 4
                    # diagonal k-tiles first so their mask-multiply overlaps
                    # the unmasked tiles' matmuls
                    kts = list(range(4 * band, nkt)) + list(range(0, 4 * band))
                    for hp in range(2):
                        pvA = psP.tile([65, ST], f32, tag="pv")
                        pvB = psP.tile([128, ST], f32, tag="pv")
                        wts = {}
                        for ii in range(nkt + slack):
                            if ii < nkt:
                                kt = kts[ii]
                                ksl = slice(kt * 128, (kt + 1) * 128)
                                j = kt - 4 * band
                                jc = max(j, 0) * 128   # cropped column offset
                                qcs = slice(qsl.start + jc, qsl.stop)
                                scp = psS.tile([128, 1024], f32, tag="sc")
                                nc.tensor.matmul(scp[:, jc:512],
                                                 kT[hp][0:64, ksl],
                                                 qT[hp][0:64, qcs],
                                                 start=True, stop=True)
                                nc.tensor.matmul(scp[:, 512 + jc:1024],
                                                 kT[hp][64:128, ksl],
                                                 qT[hp][64:128, qcs],
                                                 start=True, stop=True)
                                if pe_fill:
                                    pe_fill.pop(0)()
                                if j >= 1:
                                    wt = wt_d[j]
                                    nc.scalar.activation(wt[:, jc:512],
                                                         scp[:, jc:512],
                                                         AF.Exp, scale=0.125)
                                    nc.scalar.activation(wt[:, 512 + jc:1024],
                                                         scp[:, 512 + jc:1024],
                                                         AF.Exp, scale=0.125)
                                else:
                                    wt = wtp.tile([128, 1024], bf16, tag="wt")
                                    nc.scalar.activation(wt[:], scp[:], AF.Exp,
                                                         scale=0.125)
                                if j >= 0:
                                    meng = nc.vector
                                    meng.tensor_tensor(
                                        wt[:, jc:jc + 128],
                                        wt[:, jc:jc + 128],
                                        masks_sb[:, 0:128], Alu.mult)
                                    meng.tensor_tensor(
                                        wt[:, 512 + jc:512 + jc + 128],
                                        wt[:, 512 + jc:512 + jc + 128],
                                        masks_sb[:, 128:256], Alu.mult)
                                if debug and band == 0 and hp == 0 and ii == 0:
                                    dwt_sb = pp.tile([128, 1024], bf16, tag="dwt")
                                    nc.vector.tensor_copy(dwt_sb[:], wt[:])
                                    nc.sync.dma_start(dbg["dwt"][:], dwt_sb[:])
                                wts[ii] = (wt, jc)
                            if ii >= slack:
                                i = ii - slack
                                kt = kts[i]
                                wt, pjc = wts.pop(i)
                                nc.tensor.matmul(pvA[:, pjc:512],
                                                 va_sc[:, kt, hp, 0:VA],
                                                 wt[:, pjc:512],
                                                 start=(i == 0),
                                                 stop=(i == nkt - 1),
                                                 skip_group_check=True)
                                nc.tensor.matmul(pvB[:, pjc:512],
                                                 va_sc[:, kt, hp, VA:VHP],
                                                 wt[:, 512 + pjc:1024],
                                                 start=(i == 0),
                                                 stop=(i == nkt - 1),
                                                 skip_group_check=True)
                        # normalize: denominators at pvA row 64 (head A) and
                        # pvB row 0 (head B); broadcast bf16 copies via PE,
                        # reciprocal on the broadcast, multiply.
                        # reciprocal_approx_fast only works at partition
                        # offset 0 on hardware, so: head A broadcasts the
                        # denominator to rows 0:64 then recips; head B recips
                        # the [1,512] denominator row then broadcasts.
                        dn = dnp.tile([65, ST], bf16, tag="dn")
                        nc.vector.tensor_copy(dn[64:65, :], pvA[64:65, :])
                        rB = dnp.tile([1, ST], f32, tag="rB")
                        nc.vector.reciprocal_approx_fast(rB[:], pvB[0:1, :])
                        bcA = bcp.tile([64, ST], f32, tag="bcA")
                        bcB = bcp.tile([128, ST], bf16, tag="bcB")

                        def mk_normA(hp=hp, qsl=qsl, pvA=pvA, dn=dn, bcA=bcA):
                            bcA_ps = psO.tile([128, 512], f32, tag="oc")
                            nc.tensor.matmul(bcA_ps[0:64, :], ones_sb[64:65, 0:64],
                                             dn[64:65, :], start=True, stop=True)
                            nc.vector.reciprocal_approx_fast(bcA[:], bcA_ps[0:64, :])
                            nc.vector.tensor_tensor(concatT[hp][0:64, qsl],
                                                    pvA[0:64, :], bcA[:],
                                                    Alu.mult)
                        (mk_normA() if nofill else pe_fill.append(mk_normA))

                        def mk_normB(hp=hp, qsl=qsl, pvB=pvB, rB=rB, bcB=bcB):
                            bcB_ps = psO.tile([128, 512], f32, tag="oc")
                            nc.tensor.matmul(bcB_ps[:], ones_f32[0:1, :],
                                             rB[0:1, :], start=True, stop=True)
                            nc.vector.tensor_copy(bcB[64:128, :], bcB_ps[64:128, :])
                            nc.vector.tensor_tensor(concatT[hp][64:128, qsl],
                                                    pvB[64:128, :], bcB[64:128, :],
                                                    Alu.mult)
                        (mk_normB() if nofill else pe_fill.append(mk_normB))
                        if debug and band == 0 and hp == 0:
                            dpva_sb = pp.tile([65, 512], bf16, tag="dpva")
                            nc.vector.tensor_copy(dpva_sb[:], pvA[:])
                            nc.sync.dma_start(dbg["dpva"][:], dpva_sb[:])
                            dpvb_sb = pp.tile([128, 512], bf16, tag="dpvb")
                            nc.vector.tensor_copy(dpvb_sb[:], pvB[:])
                            nc.sync.dma_start(dbg["dpvb"][:], dpvb_sb[:])

                            def dump_bca(bcA=bcA):
                                dbca_sb = pp.tile([64, 512], bf16, tag="dbca")
                                nc.vector.tensor_copy(dbca_sb[:], bcA[:])
                                nc.sync.dma_start(dbg["dbca"][:], dbca_sb[:])
                            (dump_bca() if nofill else pe_fill.append(dump_bca))
                    for scl in range(4):
                        for ds in range(2):
                            f = emit_c_sc(band * 4 + scl, ds)
                            (f() if nofill else pe_fill.append(f))
                while pe_fill:
                    pe_fill.pop(0)()
            if debug:
                nc.sync.dma_start(dbg["dq0"][:], qT[0][:])
                nc.sync.dma_start(dbg["dk0"][:], kT[0][:])
                nc.sync.dma_start(dbg["dva"][:], v_aug[:])
                nc.sync.dma_start(dbg["dc0"][:], concatT[0][:])
                nc.sync.dma_start(dbg["dc1"][:], concatT[1][:])
    nc.finalize()
    return nc


def _rope_tables():
    inv_freq = 1.0 / (THETA ** (np.arange(0, DK, 2, dtype=np.float64) / DK))
    t = np.arange(S, dtype=np.float64)
    freqs = np.outer(t, inv_freq)
    emb = np.stack((freqs, freqs), axis=-1).reshape(S, DK)
    return np.cos(emb).astype(np.float32), np.sin(emb).astype(np.float32)


def _host_consts():
    # triangle keep-mask for the diagonal 128x128 subtile: keep k_l <= q_s
    f_idx = np.arange(128)
    p_idx = np.arange(128)
    mj = (f_idx[None, :] >= p_idx[:, None]).astype(np.float32)
    masks_np = np.tile(mj, (1, 2)).astype(ml_dtypes.bfloat16)

    vpat_np = np.zeros((128, NSC * VSC), np.float32)
    for sc in range(NSC):
        for r in range(2):
            base = sc * VSC + r * VHP
            vpat_np[:, base + 64] = 1.0   # A ones column
            vpat_np[:, base + VA] = 1.0   # B ones column
    vpat_np = vpat_np.astype(ml_dtypes.bfloat16)

    onesd_np = np.zeros((128, 128), np.float32)
    onesd_np[64, 0:64] = 1.0              # lhsT for head-A broadcast
    onesd_np[0, 64:128] = 1.0             # lhsT for head-B broadcast

    # signed pair-swap permutation: out[2i] = -q[2i+1], out[2i+1] = q[2i]
    perm_np = np.zeros((128, 128), np.float32)
    ii = np.arange(0, 128, 2)
    perm_np[ii + 1, ii] = -1.0
    perm_np[ii, ii + 1] = 1.0
    return (masks_np, vpat_np, onesd_np.astype(ml_dtypes.bfloat16),
            perm_np.astype(ml_dtypes.bfloat16))


def kernel(x, token_positions, W_q, W_k, W_v, W_o):
    global _NC
    if _NC is None:
        _NC = _build()
    x = np.asarray(x, dtype=np.float32)
    token_positions = np.asarray(token_positions)
    W_q = np.asarray(W_q, dtype=np.float32)
    W_k = np.asarray(W_k, dtype=np.float32)
    W_v = np.asarray(W_v, dtype=np.float32)
    W_o = np.asarray(W_o, dtype=np.float32)

    global _CONSTS
    if _CONSTS is None:
        _CONSTS = (*_rope_tables(), *_host_consts())
    cos_t, sin_t, masks_np, vpat_np, onesd_np, perm_np = _CONSTS

    bf = ml_dtypes.bfloat16
    in_maps = []
    for c in range(8):
        b, g = divmod(c, 4)
        rows = slice(256 * g, 256 * (g + 1))
        pw_np = np.ascontiguousarray(np.concatenate(
            [W_q[rows].T, W_k[rows].T, W_v[rows].T], axis=1)).astype(bf)
        woT_np = np.ascontiguousarray(W_o[:, rows].T).astype(bf)
        pos = np.asarray(token_positions[b], dtype=np.int64)
        cosT = np.tile(cos_t[pos].T, (2, 1))
        sinT = np.tile(sin_t[pos].T, (2, 1))
        cossin_np = np.ascontiguousarray(
            np.concatenate([cosT, sinT], axis=1)).astype(bf)
        xT_np = np.ascontiguousarray(x[b].T).astype(bf)
        in_maps.append({
            "xT": xT_np, "pw": pw_np, "woT": woT_np, "cossin": cossin_np,
            "masks": masks_np, "vpat": vpat_np, "onesd": onesd_np,
            "permd": perm_np,
        })

    res = run_bass_kernel_spmd(_NC, in_maps, core_ids=list(range(8)))
    outs = [res.results[c]["out"].astype(np.float32) for c in range(8)]
    o0 = outs[0] + outs[1] + outs[2] + outs[3]
    o1 = outs[4] + outs[5] + outs[6] + outs[7]
    return np.stack([o0, o1]).astype(np.float32)


# revision 58
# speedup vs baseline: 1.7894x; 1.0068x over previous
"""TRN2 Bass/Tile kernel: causal self-attention with RoPE.

Sharding across 8 NeuronCores: batch (2) x head-groups (4 groups of 4 heads,
tensor-parallel). Each core computes, for its batch and its 4 heads:
Q/K/V projections in bf16 (RoPE applied via a signed pair-swap permutation
matmul + cos/sin elementwise combine), causal softmax attention in transposed
(scores^T) orientation with the softmax denominator obtained via an extra
ones-column in V, and a partial output projection. The host sums the 4
partial outputs per batch.

All on-chip matmul operands are bf16 with fp32 PSUM accumulation. Phase B is
software-pipelined (scores run 2 k-tiles ahead of the PV accumulation) so the
tensor engine stays busy while the activation engine computes exp().
"""
import numpy as np
import ml_dtypes
import concourse.bass as bass
from concourse import bacc
import concourse.mybir as mybir
import concourse.tile as tile
from concourse.bass_utils import run_bass_kernel_spmd

B, S, D = 2, 2048, 1024
H, DK = 16, 64
THETA = 10000.0
ST = 512              # q-band / projection s-tile width
NSC = S // 128        # 16 s-chunks of 128
f32 = mybir.dt.float32
bf16 = mybir.dt.bfloat16
AF = mybir.ActivationFunctionType
Alu = mybir.AluOpType

# v_aug layout per s-chunk, per head pair: A head [v(64) | one],
# B head [one | zeros(63) | v(64)] (places attn rows at psum partitions 64:128)
VA = 65
VB = 128
VHP = VA + VB        # 193
VSC = 2 * VHP        # 386

_NC = None
_CONSTS = None


def _build():
    import os
    debug = bool(os.environ.get("K_DEBUG"))
    slack = int(os.environ.get("K_SLACK", "2"))
    nofill = bool(os.environ.get("K_NOFILL"))
    nc = bacc.Bacc()
    xT = nc.dram_tensor("xT", [D, S], bf16, kind="ExternalInput")
    pw = nc.dram_tensor("pw", [D, 768], bf16, kind="ExternalInput")
    woT = nc.dram_tensor("woT", [256, D], bf16, kind="ExternalInput")
    cossin = nc.dram_tensor("cossin", [128, 2 * S], bf16, kind="ExternalInput")
    masks = nc.dram_tensor("masks", [128, 256], bf16, kind="ExternalInput")
    vpat = nc.dram_tensor("vpat", [128, NSC * VSC], bf16, kind="ExternalInput")
    onesd = nc.dram_tensor("onesd", [128, 128], bf16, kind="ExternalInput")
    permd = nc.dram_tensor("permd", [128, 128], bf16, kind="ExternalInput")
    out = nc.dram_tensor("out", [S, D], bf16, kind="ExternalOutput")
    if debug:
        dbg = {name: nc.dram_tensor(name, shape, bf16, kind="ExternalOutput")
               for name, shape in [("dq0", [128, S]), ("dk0", [128, S]),
                                   ("dva", [128, NSC * VSC]),
                                   ("dc0", [128, S]), ("dc1", [128, S]),
                                   ("dwt", [128, 1024]), ("dpva", [65, 512]),
                                   ("dpvb", [128, 512]), ("dbca", [64, 512])]}

    with tile.TileContext(nc) as tc:
        with tc.tile_pool(name="persist", bufs=1) as pp:
            qT = [pp.tile([128, S], bf16, tag=f"qT{i}", name=f"qT{i}") for i in range(2)]
            kT = [pp.tile([128, S], bf16, tag=f"kT{i}", name=f"kT{i}") for i in range(2)]
            v_aug = pp.tile([128, NSC * VSC], bf16, tag="vaug")
            concatT = [pp.tile([128, S], bf16, tag=f"cT{i}", name=f"cT{i}") for i in range(2)]
            pw_sb = pp.tile([128, 8, 768], bf16, tag="pw")
            cs_sb = pp.tile([128, 2, S], bf16, tag="cs")
            woT_sb = pp.tile([128, 2, D], bf16, tag="woT")
            ones_sb = pp.tile([128, 128], bf16, tag="ones")
            perm_sb = pp.tile([128, 128], bf16, tag="perm")
            masks_sb = pp.tile([128, 256], bf16, tag="masks")
            ones_f32 = pp.tile([1, 128], f32, tag="ones_f32")

            with tc.tile_pool(name="pax", bufs=2) as pax:
                xs = [pax.tile([128, 8, ST], bf16, tag="xs", name=f"xs{i}")
                      for i in range(4)]
                # DMA order = queue order: weights + first x chunk first so the
                # PE can start ~4us in; constants later.
                nc.sync.dma_start(pw_sb[:, 0:2, :],
                                  pw[0:256, :].rearrange("(k p) m -> p k m", p=128))
                nc.sync.dma_start(xs[0][:, 0:2, :],
                                  xT[0:256, 0:ST].rearrange("(k p) m -> p k m", p=128))
                nc.sync.dma_start(pw_sb[:, 2:4, :],
                                  pw[256:512, :].rearrange("(k p) m -> p k m", p=128))
                nc.sync.dma_start(xs[0][:, 2:4, :],
                                  xT[256:512, 0:ST].rearrange("(k p) m -> p k m", p=128))
                nc.sync.dma_start(pw_sb[:, 4:8, :],
                                  pw[512:1024, :].rearrange("(k p) m -> p k m", p=128))
                nc.sync.dma_start(xs[0][:, 4:8, :],
                                  xT[512:1024, 0:ST].rearrange("(k p) m -> p k m", p=128))
                nc.sync.dma_start(perm_sb[:], permd[:])
                nc.sync.dma_start(xs[1][:],
                                  xT[:, ST:2 * ST].rearrange("(k p) m -> p k m", p=128))
                nc.sync.dma_start(cs_sb[:],
                                  cossin[:].rearrange("p (c s) -> p c s", c=2))
                nc.sync.dma_start(ones_sb[:], onesd[:])
                nc.sync.dma_start(masks_sb[:], masks[:])
                nc.sync.dma_start(woT_sb[:],
                                  woT[:].rearrange("(k p) m -> p k m", p=128))
                nc.sync.dma_start(xs[2][:],
                                  xT[:, 2 * ST:3 * ST].rearrange("(k p) m -> p k m", p=128))
                nc.sync.dma_start(xs[3][:],
                                  xT[:, 3 * ST:4 * ST].rearrange("(k p) m -> p k m", p=128))

                nc.gpsimd.memset(ones_f32[:], 1.0)
                va_sc = v_aug[:].rearrange("p (c h r) -> p c h r", c=NSC, r=VHP)
                # ones columns for the softmax denominator + zero filler
                # around them; the v blocks are overwritten by V copies.
                nc.sync.dma_start(v_aug[:], vpat[:])

                # ---- Phase A: projections + RoPE + V ----
                # Per (st, hp, q/k) tile: 8 accumulation matmuls, ACT copy of
                # the psum to SBUF, pair-swap permutation matmul (emitted one
                # tile later to hide the ACT latency), then q*cos + perm*sin
                # on DVE/Pool. For st==0 the q/k matmuls run kt-major so the
                # PE only needs the first (pw, xs) DMA chunk to start.
                with tc.tile_pool(name="pa", bufs=4, space="PSUM") as pa, \
                     tc.tile_pool(name="prot", bufs=2, space="PSUM") as prot, \
                     tc.tile_pool(name="pvps", bufs=2, space="PSUM") as pvps, \
                     tc.tile_pool(name="pqsb", bufs=3) as pqsb, \
                     tc.tile_pool(name="pt12", bufs=6) as pt12:
                    fillers = []

                    # PE p-state warm-up: keep the tensor engine busy from
                    # t~0 so the 3us continuous-execution ramp completes
                    # before the first projection matmuls arrive.
                    wz = pqsb.tile([128, 64], bf16, tag="warmz")
                    nc.gpsimd.memset(wz[:], 0.0)
                    warm_ps = pa.tile([128, ST], f32, tag="proj", name="warm")
                    for _w in range(40):
                        nc.tensor.matmul(warm_ps[0:64, 0:64], wz[:, 0:64],
                                         wz[:, 0:64], start=True, stop=True)

                    def flush_fillers():
                        while fillers:
                            fillers.pop(0)()

                    def rope_tail(st, t, ps):
                        sl = slice(st * ST, (st + 1) * ST)
                        hp, qk = divmod(t, 2)
                        dst = qT if qk == 0 else kT
                        q_sb = pqsb.tile([128, ST], bf16, tag="qsb")
                        nc.scalar.copy(q_sb[:], ps[:])
                        t1 = pt12.tile([128, ST], bf16, tag="t12")
                        nc.vector.tensor_tensor(t1[:], ps[:],
                                                cs_sb[:, 0, sl], Alu.mult)

                        def mk_perm(hp=hp, dst=dst, sl=sl, q_sb=q_sb, t1=t1):
                            rot = prot.tile([128, ST], f32, tag="rot")
                            nc.tensor.matmul(rot[:], perm_sb[:], q_sb[:],
                                             start=True, stop=True)
                            t2 = pt12.tile([128, ST], bf16, tag="t12")
                            nc.vector.tensor_tensor(t2[:], rot[:],
                                                    cs_sb[:, 1, sl], Alu.mult)
                            nc.gpsimd.tensor_tensor(dst[hp][:, sl], t1[:],
                                                    t2[:], Alu.add)
                        fillers.append(mk_perm)

                    def proj_mm(st, t, ps, kt):
                        hp, qk = divmod(t, 2)
                        off = qk * 256 + 128 * hp
                        nc.tensor.matmul(ps[:],
                                         pw_sb[:, kt, off:off + 128],
                                         xs[st][:, kt, :],
                                         start=(kt == 0), stop=(kt == 7))

                    for st in range(4):
                        if st == 0:
                            pss = [pa.tile([128, ST], f32, tag="proj",
                                           name=f"p0_{t}") for t in range(4)]
                            for kt in range(8):
                                for t in range(4):
                                    proj_mm(0, t, pss[t], kt)
                            for t in range(4):
                                flush_fillers()
                                rope_tail(0, t, pss[t])
                        else:
                            for t in range(4):
                                ps = pa.tile([128, ST], f32, tag="proj")
                                for kt in range(8):
                                    proj_mm(st, t, ps, kt)
                                flush_fillers()
                                rope_tail(st, t, ps)
                        for scl in range(4):
                            sc = st * 4 + scl
                            vp = pvps.tile([128, 256], f32, tag="vproj")
                            for kt in range(8):
                                nc.tensor.matmul(vp[:],
                                                 xs[st][:, kt, scl * 128:(scl + 1) * 128],
                                                 pw_sb[:, kt, 512:768],
                                                 start=(kt == 0), stop=(kt == 7))
                            if scl == 0:
                                flush_fillers()
                            vp_r = vp[:].rearrange("p (g t e) -> p g t e", g=2, t=2)
                            nc.scalar.copy(va_sc[:, sc, :, 0:64], vp_r[:, :, 0, :])
                            nc.scalar.copy(va_sc[:, sc, :, VA + 64:VHP],
                                           vp_r[:, :, 1, :])
                    flush_fillers()

            # ---- Phase B: attention, software-pipelined; Phase C per band ----
            with tc.tile_pool(name="psS", bufs=2, space="PSUM") as psS, \
                 tc.tile_pool(name="psP", bufs=2, space="PSUM") as psP, \
                 tc.tile_pool(name="psO", bufs=2, space="PSUM") as psO, \
                 tc.tile_pool(name="wtp", bufs=3) as wtp, \
                 tc.tile_pool(name="dnp", bufs=4) as dnp, \
                 tc.tile_pool(name="bcp", bufs=4) as bcp, \
                 tc.tile_pool(name="obp", bufs=4) as obp:
                pe_fill = []   # deferred PE work (normalize bcasts, phase C)
                # dedicated weight tiles for diagonal k-tiles j=1..3: zeroed
                # once; exp only ever rewrites the causally-needed columns,
                # so the masked-out ranges stay zero.
                wt_d = {j: pp.tile([128, 1024], bf16, tag=f"wtd{j}",
                                   name=f"wtd{j}") for j in (1, 2, 3)}
                for j in (1, 2, 3):
                    nc.gpsimd.memset(wt_d[j][:], 0.0)

                def emit_c_sc(sc, ds):
                    def emit():
                        ssl = slice(sc * 128, (sc + 1) * 128)
                        dsl = slice(ds * 512, (ds + 1) * 512)
                        op = psO.tile([128, 512], f32, tag="oc")
                        nc.tensor.matmul(op[:], concatT[0][:, ssl],
                                         woT_sb[:, 0, dsl],
                                         start=True, stop=False)
                        nc.tensor.matmul(op[:], concatT[1][:, ssl],
                                         woT_sb[:, 1, dsl],
                                         start=False, stop=True)
                        ob = obp.tile([128, 512], bf16, tag="ob")
                        nc.vector.tensor_copy(ob[:], op[:])
                        nc.sync.dma_start(out[ssl, dsl], ob[:])
                    return emit

                for band in range(4):
                    qsl = slice(band * ST, (band + 1) * ST)
                    nkt = 4 * band + 4
                    # diagonal k-tiles first so their mask-multiply overlaps
                    # the unmasked tiles' matmuls
                    kts = list(range(4 * band, nkt)) + list(range(0, 4 * band))
                    for hp in range(2):
                        pvA = psP.tile([65, ST], f32, tag="pv", name="pvA")
                        pvB = psP.tile([128, ST], f32, tag="pv", name="pvB")
                        wts = {}
                        for ii in range(nkt + slack):
                            if ii < nkt:
                                kt = kts[ii]
                                ksl = slice(kt * 128, (kt + 1) * 128)
                                j = kt - 4 * band
                                jc = max(j, 0) * 128   # cropped column offset
                                qcs = slice(qsl.start + jc, qsl.stop)
                                scp = psS.tile([128, 1024], f32, tag="sc")
                                nc.tensor.matmul(scp[:, jc:512],
                                                 kT[hp][0:64, ksl],
                                                 qT[hp][0:64, qcs],
                                                 start=True, stop=True)
                                nc.tensor.matmul(scp[:, 512 + jc:1024],
                                                 kT[hp][64:128, ksl],
                                                 qT[hp][64:128, qcs],
                                                 start=True, stop=True)
                                if pe_fill:
                                    pe_fill.pop(0)()
                                if j >= 1:
                                    wt = wt_d[j]
                                    nc.scalar.activation(wt[:, jc:512],
                                                         scp[:, jc:512],
                                                         AF.Exp, scale=0.125)
                                    nc.scalar.activation(wt[:, 512 + jc:1024],
                                                         scp[:, 512 + jc:1024],
                                                         AF.Exp, scale=0.125)
                                else:
                                    wt = wtp.tile([128, 1024], bf16, tag="wt")
                                    nc.scalar.activation(wt[:], scp[:], AF.Exp,
                                                         scale=0.125)
                                if j >= 0:
                                    meng = nc.vector
                                    meng.tensor_tensor(
                                        wt[:, jc:jc + 128],
                                        wt[:, jc:jc + 128],
                                        masks_sb[:, 0:128], Alu.mult)
                                    meng.tensor_tensor(
                                        wt[:, 512 + jc:512 + jc + 128],
                                        wt[:, 512 + jc:512 + jc + 128],
                                        masks_sb[:, 128:256], Alu.mult)
                                wts[ii] = (wt, jc)
                            if ii >= slack:
                                i = ii - slack
                                kt = kts[i]
                                wt, pjc = wts.pop(i)
                                nc.tensor.matmul(pvA[:, pjc:512],
                                                 va_sc[:, kt, hp, 0:VA],
                                                 wt[:, pjc:512],
                                                 start=(i == 0),
                                                 stop=(i == nkt - 1),
                                                 skip_group_check=True)
                                nc.tensor.matmul(pvB[:, pjc:512],
                                                 va_sc[:, kt, hp, VA:VHP],
                                                 wt[:, 512 + pjc:1024],
                                                 start=(i == 0),
                                                 stop=(i == nkt - 1),
                                                 skip_group_check=True)
                        # normalize: denominators at pvA row 64 (head A) and
                        # pvB row 0 (head B); reciprocal only at partition
                        # offset 0 (hardware ucode restriction).
                        dn = dnp.tile([65, ST], bf16, tag="dn")
                        nc.vector.tensor_copy(dn[64:65, :], pvA[64:65, :])
                        rB = dnp.tile([1, ST], f32, tag="rB")
                        nc.vector.reciprocal_approx_fast(rB[:], pvB[0:1, :])
                        bcA = bcp.tile([64, ST], f32, tag="bcA")
                        bcB = bcp.tile([128, ST], bf16, tag="bcB")

                        def mk_normA(hp=hp, qsl=qsl, pvA=pvA, dn=dn, bcA=bcA):
                            bcA_ps = psO.tile([128, 512], f32, tag="oc")
                            nc.tensor.matmul(bcA_ps[0:64, :], ones_sb[64:65, 0:64],
                                             dn[64:65, :], start=True, stop=True)
                            nc.vector.reciprocal_approx_fast(bcA[:], bcA_ps[0:64, :])
                            nc.vector.tensor_tensor(concatT[hp][0:64, qsl],
                                                    pvA[0:64, :], bcA[:],
                                                    Alu.mult)
                        (mk_normA() if nofill else pe_fill.append(mk_normA))

                        def mk_normB(hp=hp, qsl=qsl, pvB=pvB, rB=rB, bcB=bcB):
                            bcB_ps = psO.tile([128, 512], f32, tag="oc")
                            nc.tensor.matmul(bcB_ps[:], ones_f32[0:1, :],
                                             rB[0:1, :], start=True, stop=True)
                            nc.vector.tensor_copy(bcB[64:128, :], bcB_ps[64:128, :])
                            nc.vector.tensor_tensor(concatT[hp][64:128, qsl],
                                                    pvB[64:128, :], bcB[64:128, :],
                                                    Alu.mult)
                        (mk_normB() if nofill else pe_fill.append(mk_normB))
                    for scl in range(4):
                        for ds in range(2):
                            f = emit_c_sc(band * 4 + scl, ds)
                            (f() if nofill else pe_fill.append(f))
                while pe_fill:
                    pe_fill.pop(0)()
            if debug:
                nc.sync.dma_start(dbg["dq0"][:], qT[0][:])
                nc.sync.dma_start(dbg["dk0"][:], kT[0][:])
                nc.sync.dma_start(dbg["dva"][:], v_aug[:])
                nc.sync.dma_start(dbg["dc0"][:], concatT[0][:])
                nc.sync.dma_start(dbg["dc1"][:], concatT[1][:])
    nc.finalize()
    return nc


def _rope_tables():
    inv_freq = 1.0 / (THETA ** (np.arange(0, DK, 2, dtype=np.float64) / DK))
    t = np.arange(S, dtype=np.float64)
    freqs = np.outer(t, inv_freq)
    emb = np.stack((freqs, freqs), axis=-1).reshape(S, DK)
    return np.cos(emb).astype(np.float32), np.sin(emb).astype(np.float32)


def _host_consts():
    # triangle keep-mask for the diagonal 128x128 subtile: keep k_l <= q_s
    f_idx = np.arange(128)
    p_idx = np.arange(128)
    mj = (f_idx[None, :] >= p_idx[:, None]).astype(np.float32)
    masks_np = np.tile(mj, (1, 2)).astype(ml_dtypes.bfloat16)

    vpat_np = np.zeros((128, NSC * VSC), np.float32)
    for sc in range(NSC):
        for r in range(2):
            base = sc * VSC + r * VHP
            vpat_np[:, base + 64] = 1.0   # A ones column
            vpat_np[:, base + VA] = 1.0   # B ones column
    vpat_np = vpat_np.astype(ml_dtypes.bfloat16)

    onesd_np = np.zeros((128, 128), np.float32)
    onesd_np[64, 0:64] = 1.0              # lhsT for head-A broadcast
    onesd_np[0, 64:128] = 1.0             # lhsT for head-B broadcast

    # signed pair-swap permutation: out[2i] = -q[2i+1], out[2i+1] = q[2i]
    perm_np = np.zeros((128, 128), np.float32)
    ii = np.arange(0, 128, 2)
    perm_np[ii + 1, ii] = -1.0
    perm_np[ii, ii + 1] = 1.0
    return (masks_np, vpat_np, onesd_np.astype(ml_dtypes.bfloat16),
            perm_np.astype(ml_dtypes.bfloat16))


def kernel(x, token_positions, W_q, W_k, W_v, W_o):
    global _NC
    if _NC is None:
        _NC = _build()
    x = np.asarray(x, dtype=np.float32)
    token_positions = np.asarray(token_positions)
    W_q = np.asarray(W_q, dtype=np.float32)
    W_k = np.asarray(W_k, dtype=np.float32)
    W_v = np.asarray(W_v, dtype=np.float32)
    W_o = np.asarray(W_o, dtype=np.float32)

    global _CONSTS
    if _CONSTS is None:
        _CONSTS = (*_rope_tables(), *_host_consts())
    cos_t, sin_t, masks_np, vpat_np, onesd_np, perm_np = _CONSTS

    bf = ml_dtypes.bfloat16
    in_maps = []
    for c in range(8):
        b, g = divmod(c, 4)
        rows = slice(256 * g, 256 * (g + 1))
        pw_np = np.ascontiguousarray(np.concatenate(
            [W_q[rows].T, W_k[rows].T, W_v[rows].T], axis=1)).astype(bf)
        woT_np = np.ascontiguousarray(W_o[:, rows].T).astype(bf)
        pos = np.asarray(token_positions[b], dtype=np.int64)
        cosT = np.tile(cos_t[pos].T, (2, 1))
        sinT = np.tile(sin_t[pos].T, (2, 1))
        cossin_np = np.ascontiguousarray(
            np.concatenate([cosT, sinT], axis=1)).astype(bf)
        xT_np = np.ascontiguousarray(x[b].T).astype(bf)
        in_maps.append({
            "xT": xT_np, "pw": pw_np, "woT": woT_np, "cossin": cossin_np,
            "masks": masks_np, "vpat": vpat_np, "onesd": onesd_np,
            "permd": perm_np,
        })

    res = run_bass_kernel_spmd(_NC, in_maps, core_ids=list(range(8)))
    outs = [res.results[c]["out"].astype(np.float32) for c in range(8)]
    o0 = outs[0] + outs[1] + outs[2] + outs[3]
    o1 = outs[4] + outs[5] + outs[6] + outs[7]
    return np.stack([o0, o1]).astype(np.float32)


# revision 60
# speedup vs baseline: 1.8187x; 1.0164x over previous
"""TRN2 Bass/Tile kernel: causal self-attention with RoPE.

Sharding across 8 NeuronCores: batch (2) x head-groups (4 groups of 4 heads,
tensor-parallel). Each core computes, for its batch and its 4 heads:
Q/K/V projections in bf16 (RoPE applied via a signed pair-swap permutation
matmul + cos/sin elementwise combine), causal softmax attention in transposed
(scores^T) orientation with the softmax denominator obtained via an extra
ones-column in V, and a partial output projection. The host sums the 4
partial outputs per batch.

All on-chip matmul operands are bf16 with fp32 PSUM accumulation. Phase B is
software-pipelined (scores run 2 k-tiles ahead of the PV accumulation) so the
tensor engine stays busy while the activation engine computes exp().
"""
import numpy as np
import ml_dtypes
import concourse.bass as bass
from concourse import bacc
import concourse.mybir as mybir
import concourse.tile as tile
from concourse.bass_utils import run_bass_kernel_spmd

B, S, D = 2, 2048, 1024
H, DK = 16, 64
THETA = 10000.0
ST = 512              # q-band / projection s-tile width
NSC = S // 128        # 16 s-chunks of 128
f32 = mybir.dt.float32
bf16 = mybir.dt.bfloat16
AF = mybir.ActivationFunctionType
Alu = mybir.AluOpType

# v_aug layout per s-chunk, per head pair: A head [v(64) | one],
# B head [one | zeros(63) | v(64)] (places attn rows at psum partitions 64:128)
VA = 65
VB = 128
VHP = VA + VB        # 193
VSC = 2 * VHP        # 386

_NC = None
_CONSTS = None


def _build():
    import os
    debug = bool(os.environ.get("K_DEBUG"))
    slack = int(os.environ.get("K_SLACK", "2"))
    nofill = bool(os.environ.get("K_NOFILL"))
    nc = bacc.Bacc()
    xT = nc.dram_tensor("xT", [D, S], bf16, kind="ExternalInput")
    pw = nc.dram_tensor("pw", [D, 768], bf16, kind="ExternalInput")
    woT = nc.dram_tensor("woT", [256, D], bf16, kind="ExternalInput")
    cossin = nc.dram_tensor("cossin", [128, 2 * S], bf16, kind="ExternalInput")
    masks = nc.dram_tensor("masks", [128, 256], bf16, kind="ExternalInput")
    vpat = nc.dram_tensor("vpat", [128, NSC * VSC], bf16, kind="ExternalInput")
    onesd = nc.dram_tensor("onesd", [128, 128], bf16, kind="ExternalInput")
    permd = nc.dram_tensor("permd", [128, 128], bf16, kind="ExternalInput")
    out = nc.dram_tensor("out", [S, D], bf16, kind="ExternalOutput")
    if debug:
        dbg = {name: nc.dram_tensor(name, shape, bf16, kind="ExternalOutput")
               for name, shape in [("dq0", [128, S]), ("dk0", [128, S]),
                                   ("dva", [128, NSC * VSC]),
                                   ("dc0", [128, S]), ("dc1", [128, S]),
                                   ("dwt", [128, 1024]), ("dpva", [65, 512]),
                                   ("dpvb", [128, 512]), ("dbca", [64, 512])]}

    with tile.TileContext(nc) as tc:
        with tc.tile_pool(name="persist", bufs=1) as pp:
            qT = [pp.tile([128, S], bf16, tag=f"qT{i}", name=f"qT{i}") for i in range(2)]
            kT = [pp.tile([128, S], bf16, tag=f"kT{i}", name=f"kT{i}") for i in range(2)]
            v_aug = pp.tile([128, NSC * VSC], bf16, tag="vaug")
            concatT = [pp.tile([128, S], bf16, tag=f"cT{i}", name=f"cT{i}") for i in range(2)]
            pw_sb = pp.tile([128, 8, 768], bf16, tag="pw")
            cs_sb = pp.tile([128, 2, S], bf16, tag="cs")
            woT_sb = pp.tile([128, 2, D], bf16, tag="woT")
            ones_sb = pp.tile([128, 128], bf16, tag="ones")
            perm_sb = pp.tile([128, 128], bf16, tag="perm")
            masks_sb = pp.tile([128, 256], bf16, tag="masks")
            ones_f32 = pp.tile([1, 128], f32, tag="ones_f32")

            with tc.tile_pool(name="pax", bufs=2) as pax:
                xs = [pax.tile([128, 8, ST], bf16, tag="xs", name=f"xs{i}")
                      for i in range(4)]
                # DMA order = queue order: weights + first x chunk first so the
                # PE can start ~4us in; constants later.
                nc.sync.dma_start(pw_sb[:, 0:2, :],
                                  pw[0:256, :].rearrange("(k p) m -> p k m", p=128))
                nc.sync.dma_start(xs[0][:, 0:2, :],
                                  xT[0:256, 0:ST].rearrange("(k p) m -> p k m", p=128))
                nc.sync.dma_start(pw_sb[:, 2:4, :],
                                  pw[256:512, :].rearrange("(k p) m -> p k m", p=128))
                nc.sync.dma_start(xs[0][:, 2:4, :],
                                  xT[256:512, 0:ST].rearrange("(k p) m -> p k m", p=128))
                nc.sync.dma_start(pw_sb[:, 4:8, :],
                                  pw[512:1024, :].rearrange("(k p) m -> p k m", p=128))
                nc.sync.dma_start(xs[0][:, 4:8, :],
                                  xT[512:1024, 0:ST].rearrange("(k p) m -> p k m", p=128))
                nc.sync.dma_start(perm_sb[:], permd[:])
                nc.sync.dma_start(xs[1][:],
                                  xT[:, ST:2 * ST].rearrange("(k p) m -> p k m", p=128))
                nc.sync.dma_start(cs_sb[:],
                                  cossin[:].rearrange("p (c s) -> p c s", c=2))
                nc.sync.dma_start(ones_sb[:], onesd[:])
                nc.sync.dma_start(masks_sb[:], masks[:])
                nc.sync.dma_start(woT_sb[:],
                                  woT[:].rearrange("(k p) m -> p k m", p=128))
                nc.sync.dma_start(xs[2][:],
                                  xT[:, 2 * ST:3 * ST].rearrange("(k p) m -> p k m", p=128))
                nc.sync.dma_start(xs[3][:],
                                  xT[:, 3 * ST:4 * ST].rearrange("(k p) m -> p k m", p=128))

                nc.gpsimd.memset(ones_f32[:], 1.0)
                va_sc = v_aug[:].rearrange("p (c h r) -> p c h r", c=NSC, r=VHP)
                # ones columns for the softmax denominator + zero filler
                # around them; the v blocks are overwritten by V copies.
                nc.sync.dma_start(v_aug[:], vpat[:])

                # ---- Phase A: projections + RoPE + V ----
                # Per (st, hp, q/k) tile: 8 accumulation matmuls, ACT copy of
                # the psum to SBUF, pair-swap permutation matmul (emitted one
                # tile later to hide the ACT latency), then q*cos + perm*sin
                # on DVE/Pool. For st==0 the q/k matmuls run kt-major so the
                # PE only needs the first (pw, xs) DMA chunk to start.
                with tc.tile_pool(name="pa", bufs=4, space="PSUM") as pa, \
                     tc.tile_pool(name="prot", bufs=2, space="PSUM") as prot, \
                     tc.tile_pool(name="pvps", bufs=2, space="PSUM") as pvps, \
                     tc.tile_pool(name="pqsb", bufs=3) as pqsb, \
                     tc.tile_pool(name="pt12", bufs=6) as pt12:
                    fillers = []

                    # PE p-state warm-up: keep the tensor engine busy from
                    # t~0 so the 3us continuous-execution ramp completes
                    # before the first projection matmuls arrive.
                    wz = pqsb.tile([128, 64], bf16, tag="warmz")
                    nc.gpsimd.memset(wz[:], 0.0)
                    warm_ps = pa.tile([128, ST], f32, tag="proj", name="warm")
                    for _w in range(40):
                        nc.tensor.matmul(warm_ps[0:64, 0:64], wz[:, 0:64],
                                         wz[:, 0:64], start=True, stop=True)

                    def flush_fillers():
                        while fillers:
                            fillers.pop(0)()

                    def rope_tail(st, t, ps):
                        sl = slice(st * ST, (st + 1) * ST)
                        hp, qk = divmod(t, 2)
                        dst = qT if qk == 0 else kT
                        q_sb = pqsb.tile([128, ST], bf16, tag="qsb")
                        nc.scalar.copy(q_sb[:], ps[:])
                        t1 = pt12.tile([128, ST], bf16, tag="t12")
                        nc.vector.tensor_tensor(t1[:], ps[:],
                                                cs_sb[:, 0, sl], Alu.mult)

                        def mk_perm(hp=hp, dst=dst, sl=sl, q_sb=q_sb, t1=t1):
                            rot = prot.tile([128, ST], f32, tag="rot")
                            nc.tensor.matmul(rot[:], perm_sb[:], q_sb[:],
                                             start=True, stop=True)
                            t2 = pt12.tile([128, ST], bf16, tag="t12")
                            nc.vector.tensor_tensor(t2[:], rot[:],
                                                    cs_sb[:, 1, sl], Alu.mult)
                            nc.gpsimd.tensor_tensor(dst[hp][:, sl], t1[:],
                                                    t2[:], Alu.add)
                        fillers.append(mk_perm)

                    def proj_mm(st, t, ps, kt):
                        hp, qk = divmod(t, 2)
                        off = qk * 256 + 128 * hp
                        nc.tensor.matmul(ps[:],
                                         pw_sb[:, kt, off:off + 128],
                                         xs[st][:, kt, :],
                                         start=(kt == 0), stop=(kt == 7))

                    for st in range(4):
                        if st == 0:
                            pss = [pa.tile([128, ST], f32, tag="proj",
                                           name=f"p0_{t}") for t in range(4)]
                            for kt in range(8):
                                for t in range(4):
                                    proj_mm(0, t, pss[t], kt)
                            for t in range(4):
                                flush_fillers()
                                rope_tail(0, t, pss[t])
                        else:
                            for t in range(4):
                                ps = pa.tile([128, ST], f32, tag="proj")
                                for kt in range(8):
                                    proj_mm(st, t, ps, kt)
                                flush_fillers()
                                rope_tail(st, t, ps)
                        for scl in range(4):
                            sc = st * 4 + scl
                            vp = pvps.tile([128, 256], f32, tag="vproj")
                            for kt in range(8):
                                nc.tensor.matmul(vp[:],
                                                 xs[st][:, kt, scl * 128:(scl + 1) * 128],
                                                 pw_sb[:, kt, 512:768],
                                                 start=(kt == 0), stop=(kt == 7))
                            if scl == 0:
                                flush_fillers()
                            vp_r = vp[:].rearrange("p (g t e) -> p g t e", g=2, t=2)
                            nc.scalar.copy(va_sc[:, sc, :, 0:64], vp_r[:, :, 0, :])
                            nc.scalar.copy(va_sc[:, sc, :, VA + 64:VHP],
                                           vp_r[:, :, 1, :])
                    flush_fillers()

            # ---- Phase B: attention, software-pipelined; Phase C per band ----
            with tc.tile_pool(name="psS", bufs=2, space="PSUM") as psS, \
                 tc.tile_pool(name="psP", bufs=2, space="PSUM") as psP, \
                 tc.tile_pool(name="psO", bufs=2, space="PSUM") as psO, \
                 tc.tile_pool(name="wtp", bufs=3) as wtp, \
                 tc.tile_pool(name="dnp", bufs=4) as dnp, \
                 tc.tile_pool(name="bcp", bufs=4) as bcp, \
                 tc.tile_pool(name="obp", bufs=4) as obp:
                pe_fill = []   # deferred PE work (normalize bcasts, phase C)
                # dedicated weight tiles for diagonal k-tiles j=1..3: zeroed
                # once; exp only ever rewrites the causally-needed columns,
                # so the masked-out ranges stay zero.
                wt_d = {j: pp.tile([128, 1024], bf16, tag=f"wtd{j}",
                                   name=f"wtd{j}") for j in (1, 2, 3)}
                for j in (1, 2, 3):
                    nc.gpsimd.memset(wt_d[j][:], 0.0)

                def emit_c_sc(sc, ds):
                    def emit():
                        ssl = slice(sc * 128, (sc + 1) * 128)
                        dsl = slice(ds * 512, (ds + 1) * 512)
                        op = psO.tile([128, 512], f32, tag="oc")
                        nc.tensor.matmul(op[:], concatT[0][:, ssl],
                                         woT_sb[:, 0, dsl],
                                         start=True, stop=False)
                        nc.tensor.matmul(op[:], concatT[1][:, ssl],
                                         woT_sb[:, 1, dsl],
                                         start=False, stop=True)
                        ob = obp.tile([128, 512], bf16, tag="ob")
                        nc.vector.tensor_copy(ob[:], op[:])
                        nc.sync.dma_start(out[ssl, dsl], ob[:])
                    return emit

                for band in range(4):
                    qsl = slice(band * ST, (band + 1) * ST)
                    nkt = 4 * band + 4
                    # diagonal k-tiles first so their mask-multiply overlaps
                    # the unmasked tiles' matmuls
                    kts = list(range(4 * band, nkt)) + list(range(0, 4 * band))
                    for hp in range(2):
                        pvA = psP.tile([65, ST], f32, tag="pv", name="pvA")
                        pvB = psP.tile([128, ST], f32, tag="pv", name="pvB")
                        wts = {}
                        for ii in range(nkt + slack):
                            if ii < nkt:
                                kt = kts[ii]
                                ksl = slice(kt * 128, (kt + 1) * 128)
                                j = kt - 4 * band
                                jc = max(j, 0) * 128   # cropped column offset
                                qcs = slice(qsl.start + jc, qsl.stop)
                                scp = psS.tile([128, 1024], f32, tag="sc")
                                nc.tensor.matmul(scp[:, jc:512],
                                                 kT[hp][0:64, ksl],
                                                 qT[hp][0:64, qcs],
                                                 start=True, stop=True)
                                nc.tensor.matmul(scp[:, 512 + jc:1024],
                                                 kT[hp][64:128, ksl],
                                                 qT[hp][64:128, qcs],
                                                 start=True, stop=True)
                                if pe_fill:
                                    pe_fill.pop(0)()
                                if j >= 1:
                                    wt = wt_d[j]
                                    wtv = wt[:].rearrange("p (h c) -> p h c", h=2)
                                    scv = scp[:].rearrange("p (h c) -> p h c", h=2)
                                    nc.scalar.activation(wtv[:, :, jc:512],
                                                         scv[:, :, jc:512],
                                                         AF.Exp, scale=0.125)
                                else:
                                    wt = wtp.tile([128, 1024], bf16, tag="wt")
                                    nc.scalar.activation(wt[:], scp[:], AF.Exp,
                                                         scale=0.125)
                                if j >= 0:
                                    wtm = wt[:].rearrange("p (h c) -> p h c", h=2)
                                    mkm = masks_sb[:].rearrange("p (h c) -> p h c", h=2)
                                    nc.vector.tensor_tensor(
                                        wtm[:, :, jc:jc + 128],
                                        wtm[:, :, jc:jc + 128],
                                        mkm[:], Alu.mult)
                                wts[ii] = (wt, jc)
                            if ii >= slack:
                                i = ii - slack
                                kt = kts[i]
                                wt, pjc = wts.pop(i)
                                nc.tensor.matmul(pvA[:, pjc:512],
                                                 va_sc[:, kt, hp, 0:VA],
                                                 wt[:, pjc:512],
                                                 start=(i == 0),
                                                 stop=(i == nkt - 1),
                                                 skip_group_check=True)
                                nc.tensor.matmul(pvB[:, pjc:512],
                                                 va_sc[:, kt, hp, VA:VHP],
                                                 wt[:, 512 + pjc:1024],
                                                 start=(i == 0),
                                                 stop=(i == nkt - 1),
                                                 skip_group_check=True)
                        # normalize: denominators at pvA row 64 (head A) and
                        # pvB row 0 (head B); reciprocal only at partition
                        # offset 0 (hardware ucode restriction).
                        dn = dnp.tile([65, ST], bf16, tag="dn")
                        nc.vector.tensor_copy(dn[64:65, :], pvA[64:65, :])
                        rB = dnp.tile([1, ST], f32, tag="rB")
                        nc.vector.reciprocal_approx_fast(rB[:], pvB[0:1, :])
                        bcA = bcp.tile([64, ST], f32, tag="bcA")
                        bcB = bcp.tile([128, ST], bf16, tag="bcB")

                        def mk_normA(hp=hp, qsl=qsl, pvA=pvA, dn=dn, bcA=bcA):
                            bcA_ps = psO.tile([128, 512], f32, tag="oc")
                            nc.tensor.matmul(bcA_ps[0:64, :], ones_sb[64:65, 0:64],
                                             dn[64:65, :], start=True, stop=True)
                            nc.vector.reciprocal_approx_fast(bcA[:], bcA_ps[0:64, :])
                            nc.vector.tensor_tensor(concatT[hp][0:64, qsl],
                                                    pvA[0:64, :], bcA[:],
                                                    Alu.mult)
                        (mk_normA() if nofill else pe_fill.append(mk_normA))

                        def mk_normB(hp=hp, qsl=qsl, pvB=pvB, rB=rB, bcB=bcB):
                            bcB_ps = psO.tile([128, 512], f32, tag="oc")
                            nc.tensor.matmul(bcB_ps[:], ones_f32[0:1, :],
                                             rB[0:1, :], start=True, stop=True)
                            nc.vector.tensor_copy(bcB[64:128, :], bcB_ps[64:128, :])
                            nc.vector.tensor_tensor(concatT[hp][64:128, qsl],
                                                    pvB[64:128, :], bcB[64:128, :],
                                                    Alu.mult)
                        (mk_normB() if nofill else pe_fill.append(mk_normB))
                    for scl in range(4):
                        for ds in range(2):
                            f = emit_c_sc(band * 4 + scl, ds)
                            (f() if nofill else pe_fill.append(f))
                while pe_fill:
                    pe_fill.pop(0)()
            if debug:
                nc.sync.dma_start(dbg["dq0"][:], qT[0][:])
                nc.sync.dma_start(dbg["dk0"][:], kT[0][:])
                nc.sync.dma_start(dbg["dva"][:], v_aug[:])
                nc.sync.dma_start(dbg["dc0"][:], concatT[0][:])
                nc.sync.dma_start(dbg["dc1"][:], concatT[1][:])
    nc.finalize()
    return nc


def _rope_tables():
    inv_freq = 1.0 / (THETA ** (np.arange(0, DK, 2, dtype=np.float64) / DK))
    t = np.arange(S, dtype=np.float64)
    freqs = np.outer(t, inv_freq)
    emb = np.stack((freqs, freqs), axis=-1).reshape(S, DK)
    return np.cos(emb).astype(np.float32), np.sin(emb).astype(np.float32)


def _host_consts():
    # triangle keep-mask for the diagonal 128x128 subtile: keep k_l <= q_s
    f_idx = np.arange(128)
    p_idx = np.arange(128)
    mj = (f_idx[None, :] >= p_idx[:, None]).astype(np.float32)
    masks_np = np.tile(mj, (1, 2)).astype(ml_dtypes.bfloat16)

    vpat_np = np.zeros((128, NSC * VSC), np.float32)
    for sc in range(NSC):
        for r in range(2):
            base = sc * VSC + r * VHP
            vpat_np[:, base + 64] = 1.0   # A ones column
            vpat_np[:, base + VA] = 1.0   # B ones column
    vpat_np = vpat_np.astype(ml_dtypes.bfloat16)

    onesd_np = np.zeros((128, 128), np.float32)
    onesd_np[64, 0:64] = 1.0              # lhsT for head-A broadcast
    onesd_np[0, 64:128] = 1.0             # lhsT for head-B broadcast

    # signed pair-swap permutation: out[2i] = -q[2i+1], out[2i+1] = q[2i]
    perm_np = np.zeros((128, 128), np.float32)
    ii = np.arange(0, 128, 2)
    perm_np[ii + 1, ii] = -1.0
    perm_np[ii, ii + 1] = 1.0
    return (masks_np, vpat_np, onesd_np.astype(ml_dtypes.bfloat16),
            perm_np.astype(ml_dtypes.bfloat16))


def kernel(x, token_positions, W_q, W_k, W_v, W_o):
    global _NC
    if _NC is None:
        _NC = _build()
    x = np.asarray(x, dtype=np.float32)
    token_positions = np.asarray(token_positions)
    W_q = np.asarray(W_q, dtype=np.float32)
    W_k = np.asarray(W_k, dtype=np.float32)
    W_v = np.asarray(W_v, dtype=np.float32)
    W_o = np.asarray(W_o, dtype=np.float32)

    global _CONSTS
    if _CONSTS is None:
        _CONSTS = (*_rope_tables(), *_host_consts())
    cos_t, sin_t, masks_np, vpat_np, onesd_np, perm_np = _CONSTS

    bf = ml_dtypes.bfloat16
    in_maps = []
    for c in range(8):
        b, g = divmod(c, 4)
        rows = slice(256 * g, 256 * (g + 1))
        pw_np = np.ascontiguousarray(np.concatenate(
            [W_q[rows].T, W_k[rows].T, W_v[rows].T], axis=1)).astype(bf)
        woT_np = np.ascontiguousarray(W_o[:, rows].T).astype(bf)
        pos = np.asarray(token_positions[b], dtype=np.int64)
        cosT = np.tile(cos_t[pos].T, (2, 1))
        sinT = np.tile(sin_t[pos].T, (2, 1))
        cossin_np = np.ascontiguousarray(
            np.concatenate([cosT, sinT], axis=1)).astype(bf)
        xT_np = np.ascontiguousarray(x[b].T).astype(bf)
        in_maps.append({
            "xT": xT_np, "pw": pw_np, "woT": woT_np, "cossin": cossin_np,
            "masks": masks_np, "vpat": vpat_np, "onesd": onesd_np,
            "permd": perm_np,
        })

    res = run_bass_kernel_spmd(_NC, in_maps, core_ids=list(range(8)))
    outs = [res.results[c]["out"].astype(np.float32) for c in range(8)]
    o0 = outs[0] + outs[1] + outs[2] + outs[3]
    o1 = outs[4] + outs[5] + outs[6] + outs[7]
    return np.stack([o0, o1]).astype(np.float32)
